# revision 15
# baseline (speedup 1.0000x reference)
"""Trainium2 Bass kernel for nn_Adiin_24197845746021 (gnn_message_passing).

Row-shard the N=4096 nodes across 8 NeuronCores (512 rows each). Each core
holds adj[rows,:].T (bf16) resident in SBUF. adj @ X is computed as
lhsT = X_full tile (AllGathered, node-major), rhs = adjT shard ->
feature-major output. Feature matmuls consume feature-major activations
directly, so no transposes are needed except for the tiny [*,10] tensors.
All matmuls bf16 (fp32 PSUM); gating / softmax / student-t math fp32.
Contractions are zero-padded to multiples of 128 host-side. Wide (2000-dim)
mid-lived activations (z3, h3, dz1) spill to DRAM and are re-streamed.
"""

import numpy as np
import ml_dtypes

import concourse.bass as bass
import concourse.mybir as mybir
import concourse.tile as tile
from concourse import bacc
from concourse.bass_utils import run_bass_kernel_spmd
from concourse.masks import make_identity

BF = mybir.dt.bfloat16
F32 = mybir.dt.float32
AF = mybir.ActivationFunctionType
ALU = mybir.AluOpType
bf16 = ml_dtypes.bfloat16

P = 128
NCORES = 8
N_FULL = 4096
NIN, E1, E3, NZ, K = 2000, 500, 2000, 10, 10
NINP, EP, E3P, NZP = 2048, 512, 2048, 16       # padded dims
NIN_CH, E_CH, E3_CH = NINP // P, EP // P, E3P // P   # 16, 4, 16
CATP = EP + EP + E3P + P                        # 3200: [z1|z2|z3|z] padded
CAT_CH = CATP // P                              # 25

_BUILD_CACHE = {}


def build_graph(N=N_FULL):
    R = N // NCORES          # rows per core
    NB = R // P              # node blocks per core
    CC = N // P              # contraction (column) chunks for spmm

    nc = bacc.Bacc("TRN2", target_bir_lowering=False, debug=False,
                   num_devices=NCORES)

    def din(name, shape, dt=BF):
        return nc.declare_dram_parameter(name, list(shape), dt, isOutput=False)

    def dout(name, shape):
        return nc.declare_dram_parameter(name, list(shape), F32, isOutput=True)

    adjT_p = din("adjT", [N, R])
    xT_p = din("xT", [NINP, R])
    wp = {}
    for nm, sh in [
        ("enc1_w", (NINP, EP)), ("enc2_w", (EP, EP)), ("enc3_w", (EP, E3P)),
        ("zl_w", (E3P, NZP)), ("dec1_w", (P, NINP)), ("dec2_w", (NINP, EP)),
        ("dec3_w", (EP, EP)), ("xbar_w", (EP, NINP)),
        ("g1_w", (NINP, EP)), ("g2_w", (EP, EP)), ("g3_w", (EP, E3P)),
        ("g4_w", (E3P, NZP)), ("g6_w", (P, NINP)), ("g7_w", (NINP, EP)),
        ("g8_w", (EP, EP)), ("g9_w", (EP, NINP)),
        ("agnn_w", (CATP, NZP)), ("m1_w", (2 * EP, 2)), ("m2_w", (2 * EP, 2)),
        ("m3_w", (2 * E3P, 2)), ("ml_w", (CATP, 4)), ("wq", (P, NZP)),
    ]:
        wp[nm] = din(nm, sh)
    bp = {}
    for nm, sh in [
        ("enc1_b", (P, E_CH)), ("enc2_b", (P, E_CH)), ("enc3_b", (P, E3_CH)),
        ("zl_b", (NZP, 1)), ("dec1_b", (P, NIN_CH)), ("dec2_b", (P, E_CH)),
        ("dec3_b", (P, E_CH)), ("xbar_b", (P, NIN_CH)),
        ("m1_b", (2, 1)), ("m2_b", (2, 1)), ("m3_b", (2, 1)), ("ml_b", (4, 1)),
    ]:
        bp[nm] = din(nm, sh, F32)

    o_xbarT = dout("o_xbarT", [NINP, R])
    o_zhatT = dout("o_zhatT", [NINP, R])
    o_adjhat = dout("o_adjhat", [R, N])
    o_small = {nm: dout("o_" + nm, [R, NZP])
               for nm in ("q", "q1", "z", "r", "zl", "pred")}

    rg = [list(range(NCORES))]

    from contextlib import ExitStack
    with tile.TileContext(nc) as tc, ExitStack() as stack:
        pers = stack.enter_context(tc.tile_pool(name="pers", bufs=1))
        wst = stack.enter_context(tc.tile_pool(name="wst", bufs=3))
        tst = stack.enter_context(tc.tile_pool(name="tst", bufs=4))
        tmp = stack.enter_context(tc.tile_pool(name="tmp", bufs=2))
        sbst = stack.enter_context(tc.tile_pool(name="sbst", bufs=3))
        tlocp = stack.enter_context(tc.tile_pool(name="tlocp", bufs=1))
        ps = stack.enter_context(tc.tile_pool(name="ps", bufs=8, space="PSUM"))
        dram = stack.enter_context(tc.tile_pool(name="dram", bufs=1, space="DRAM"))

        _nmc = [0]

        def uname(pfx):
            _nmc[0] += 1
            return f"{pfx}{_nmc[0]}"

        # ---------------- constants / params in SBUF ----------------
        ident = pers.tile([P, P], BF, tag="ident")
        make_identity(nc, ident[:])
        ones_f = pers.tile([P, 1], F32, tag="ones_f")
        nc.gpsimd.memset(ones_f[:], 1.0)
        ones_row = pers.tile([1, 512], BF, tag="ones_row")
        nc.gpsimd.memset(ones_row[:], 1.0)

        adjT = pers.tile([P, CC, R], BF, tag="adjT")
        nc.sync.dma_start(adjT[:], adjT_p.ap().rearrange("(po pi) f -> pi po f", pi=P))

        bias = {}
        for nm in bp:
            t = pers.tile(list(bp[nm].shape), F32, tag="b_" + nm, name="b_" + nm)
            nc.sync.dma_start(t[:], bp[nm].ap())
            bias[nm] = t

        def resident_w(nm, nch, nl):
            t = pers.tile([P, nch, nl], BF, tag="w_" + nm, name="w_" + nm)
            nc.sync.dma_start(t[:], wp[nm].ap().rearrange("(po pi) l -> pi po l", pi=P))
            return t

        m1w = resident_w("m1_w", 8, 2)
        m2w = resident_w("m2_w", 8, 2)
        m3w = resident_w("m3_w", 32, 2)
        mlw = resident_w("ml_w", CAT_CH, 4)
        agnnw = resident_w("agnn_w", CAT_CH, NZP)
        zlw = resident_w("zl_w", E3_CH, NZP)
        g4w = resident_w("g4_w", E3_CH, NZP)
        wqw = resident_w("wq", 1, NZP)
        dec1w = resident_w("dec1_w", 1, NINP)
        g6w = resident_w("g6_w", 1, NINP)

        # ---------------- helpers ----------------
        def wtile(param, k, f0, fsz):
            t = wst.tile([P, fsz], BF, tag="w", name=uname("wt"))
            nc.sync.dma_start(t[:], param.ap()[k * P:(k + 1) * P, f0:f0 + fsz])
            return t

        def psum(shape, dt=F32):
            return ps.tile(list(shape), dt, tag="ps", name=uname("ps"))

        def sb_chunk(src_dram, c):
            """stream one feature-major [128, R] bf16 chunk back from DRAM"""
            t = sbst.tile([P, R], BF, tag="sb", name=uname("sb"))
            nc.sync.dma_start(t[:], src_dram[c * P:(c + 1) * P, :])
            return t[:]

        def fmajor_linear(dst, src_fn, wparam, nk, nfo_total, bias_t=None,
                          act=AF.Relu, dst_dram=None):
            """feature-major linear: out = act(W.T @ src + b).
            dst: [128, nfo, R] sbuf or None; dst_dram: [nfo*128, R] dram spill.
            src_fn(k) -> [128, R] AP."""
            nfo = nfo_total // P
            for g0 in range(0, nfo, 4):
                gn = min(4, nfo - g0)
                pss = [psum([P, R]) for _ in range(gn)]
                for k_ in range(nk):
                    wt = wtile(wparam, k_, g0 * P, gn * P)
                    src = src_fn(k_)
                    for j in range(gn):
                        nc.tensor.matmul(pss[j][:], wt[:, j * P:(j + 1) * P],
                                         src, start=(k_ == 0), stop=(k_ == nk - 1))
                for j in range(gn):
                    fo = g0 + j
                    b_ap = bias_t[:, fo:fo + 1] if bias_t is not None else 0.0
                    if dst is not None:
                        nc.scalar.activation(dst[:, fo, :], pss[j][:], act, bias=b_ap)
                    if dst_dram is not None:
                        ot = sbst.tile([P, R], BF, tag="sb", name=uname("fo"))
                        nc.scalar.activation(ot[:], pss[j][:], act, bias=b_ap)
                        nc.sync.dma_start(dst_dram[fo * P:(fo + 1) * P, :], ot[:])

        def nmajor_linear(dst, src_fn, wparam, nk, fo_total, wtiles=None):
            """node-major inner product: dst [128, NB, fo_total] = src.T @ W.
            src_fn(k) -> feature-major [128, R] chunk."""
            for fog in range((fo_total + 511) // 512):
                fsz = min(512, fo_total - fog * 512)
                pss = [psum([P, fsz]) for _ in range(NB)]
                for k_ in range(nk):
                    if wtiles is not None:
                        wt = wtiles[:, k_, fog * 512:fog * 512 + fsz]
                    else:
                        wt = wtile(wparam, k_, fog * 512, fsz)[:]
                    src = src_fn(k_)
                    for nb in range(NB):
                        nc.tensor.matmul(pss[nb][:], src[:, nb * P:(nb + 1) * P],
                                         wt, start=(k_ == 0), stop=(k_ == nk - 1))
                for nb in range(NB):
                    nc.vector.tensor_copy(dst[:, nb, fog * 512:fog * 512 + fsz],
                                          pss[nb][:])

        def allgather(loc, w, name):
            """loc [128, NB, w] bf16 node-major -> gathered [N, w] dram."""
            bounce = dram.tile([R, w], BF, tag="b_" + name, name="b_" + name)
            nc.sync.dma_start(bounce[:].rearrange("(po pi) f -> pi po f", pi=P), loc[:])
            full = dram.tile([N, w], BF, tag="g_" + name, name="g_" + name,
                             addr_space="Shared")
            nc.gpsimd.collective_compute(
                "AllGather", ALU.bypass, replica_groups=rg,
                ins=[bounce[:].opt()], outs=[full[:].opt()])
            return full

        def spmm_f(dstT, tfull, w_total, act, dst_dram=None, fp32_dram=None):
            """feature-major spmm: dst = act(adj_rows @ tfull).T chunks."""
            nf = w_total // P
            for g0 in range(0, nf, 8):
                gn = min(8, nf - g0)
                pss = [psum([P, R]) for _ in range(gn)]
                for col in range(CC):
                    tt = tst.tile([P, gn * P], BF, tag="t", name=uname("t"))
                    nc.sync.dma_start(tt[:], tfull[col * P:(col + 1) * P,
                                                   g0 * P:(g0 + gn) * P])
                    for j in range(gn):
                        nc.tensor.matmul(pss[j][:], tt[:, j * P:(j + 1) * P],
                                         adjT[:, col, :], start=(col == 0),
                                         stop=(col == CC - 1))
                for j in range(gn):
                    fo = g0 + j
                    if fp32_dram is not None:
                        xo = tmp.tile([P, R], F32, tag="fo32", name=uname("fo32"),
                                      bufs=2)
                        nc.scalar.activation(xo[:], pss[j][:], act)
                        nc.sync.dma_start(fp32_dram[fo * P:(fo + 1) * P, :], xo[:])
                        if dstT is not None:
                            nc.vector.tensor_copy(dstT[:, fo, :], xo[:])
                        continue
                    if dstT is not None:
                        nc.scalar.activation(dstT[:, fo, :], pss[j][:], act)
                    if dst_dram is not None:
                        ot = sbst.tile([P, R], BF, tag="sb", name=uname("so"))
                        nc.scalar.activation(ot[:], pss[j][:], act)
                        nc.sync.dma_start(dst_dram[fo * P:(fo + 1) * P, :], ot[:])

        def spmm_narrow(tfull):
            pp = psum([NZP, R])
            for col in range(CC):
                tt = tst.tile([P, NZP], BF, tag="tn", name=uname("tn"))
                nc.sync.dma_start(tt[:], tfull[col * P:(col + 1) * P, :])
                nc.tensor.matmul(pp[:], tt[:], adjT[:, col, :],
                                 start=(col == 0), stop=(col == CC - 1))
            return pp

        def gate(src_list, wres, nktot, bias_t, nl):
            """softmax(lrelu(cat @ w + b)) row-L2-normalized.
            src_list: [(chunk_fn, nch)]; returns nl fp32 [128,R] bcast tiles."""
            pp = psum([nl, R])
            k_ = 0
            for fn, nch in src_list:
                for c in range(nch):
                    nc.tensor.matmul(pp[:], wres[:, k_, :nl], fn(c),
                                     start=(k_ == 0), stop=(k_ == nktot - 1))
                    k_ += 1
            assert k_ == nktot
            e_full = tmp.tile([P, R], F32, tag="gt_e", name=uname("gt_e"), bufs=1)
            nc.vector.memset(e_full[:], 0.0)
            lin = tmp.tile([nl, R], F32, tag="gt_lin", name=uname("gt_lin"), bufs=1)
            nc.scalar.activation(lin[:], pp[:], AF.Identity, bias=bias_t[:, 0:1])
            sc = tmp.tile([nl, R], F32, tag="gt_sc", name=uname("gt_sc"), bufs=1)
            nc.vector.tensor_scalar_mul(sc[:], lin[:], 0.01)
            lr = tmp.tile([nl, R], F32, tag="gt_lr", name=uname("gt_lr"), bufs=1)
            nc.vector.tensor_max(lr[:], lin[:], sc[:])
            nc.scalar.activation(e_full[0:nl, :], lr[:], AF.Exp)
            s1 = psum([1, R])
            nc.tensor.matmul(s1[:], ones_f[:, 0:1], e_full[:], start=True, stop=True)
            rinv = tmp.tile([1, R], F32, tag="gt_rinv", name=uname("gt_rinv"), bufs=1)
            nc.vector.reciprocal(rinv[:], s1[:])
            rb = tmp.tile([P, R], F32, tag="gt_rb", name=uname("gt_rb"), bufs=1)
            nc.gpsimd.partition_broadcast(rb[:], rinv[:])
            p_t = tmp.tile([P, R], F32, tag="gt_p", name=uname("gt_p"), bufs=1)
            nc.vector.memset(p_t[:], 0.0)
            nc.vector.tensor_mul(p_t[0:nl, :], e_full[0:nl, :], rb[0:nl, :])
            sq = tmp.tile([P, R], F32, tag="gt_sq", name=uname("gt_sq"), bufs=1)
            nc.vector.memset(sq[:], 0.0)
            nc.vector.tensor_mul(sq[0:nl, :], p_t[0:nl, :], p_t[0:nl, :])
            n2 = psum([1, R])
            nc.tensor.matmul(n2[:], ones_f[:, 0:1], sq[:], start=True, stop=True)
            sqr = tmp.tile([1, R], F32, tag="gt_sqr", name=uname("gt_sqr"), bufs=1)
            nc.scalar.activation(sqr[:], n2[:], AF.Sqrt)
            rn = tmp.tile([1, R], F32, tag="gt_rn", name=uname("gt_rn"), bufs=1)
            nc.vector.reciprocal(rn[:], sqr[:])
            rnb = tmp.tile([P, R], F32, tag="gt_rnb", name=uname("gt_rnb"), bufs=1)
            nc.gpsimd.partition_broadcast(rnb[:], rn[:])
            nc.vector.tensor_mul(p_t[0:nl, :], p_t[0:nl, :], rnb[0:nl, :])
            outs = []
            for i in range(nl):
                row = tmp.tile([1, R], F32, tag=f"gt_row{i}", name=uname("gt_row"),
                               bufs=1)
                if i == 0:
                    nc.vector.tensor_copy(row[:], p_t[0:1, :])
                else:
                    nc.sync.dma_start(row[:], p_t[i:i + 1, :])
                pb = tmp.tile([P, R], F32, tag=f"gt_pb{i}", name=uname("gt_pb"),
                              bufs=1)
                nc.gpsimd.partition_broadcast(pb[:], row[:])
                outs.append(pb)
            return outs

        def mix_chunk(a_ap, b_ap, pa, pb_, out_ap=None):
            """one [128,R] bf16 chunk of pa*a + pb*b"""
            t0 = tmp.tile([P, R], F32, tag="mx0", name=uname("mx0"))
            t1 = tmp.tile([P, R], F32, tag="mx1", name=uname("mx1"))
            if out_ap is None:
                out_ap = tmp.tile([P, R], BF, tag="mxo", name=uname("mxo"))[:]
            nc.vector.tensor_mul(t0[:], a_ap, pa[:])
            nc.vector.tensor_mul(t1[:], b_ap, pb_[:])
            nc.vector.tensor_add(out_ap, t0[:], t1[:])
            return out_ap

        def narrow_nm(srcT, bf_dst=None, f32_dram=None, want_f32=False):
            f32s = []
            for nb in range(NB):
                pt = psum([P, NZP], BF)
                nc.tensor.transpose(pt[:], srcT[0:NZP, nb * P:(nb + 1) * P],
                                    ident[0:NZP, 0:NZP])
                if bf_dst is not None:
                    nc.vector.tensor_copy(bf_dst[:, nb, :], pt[:])
                ft = None
                if want_f32 or f32_dram is not None:
                    ft = tmp.tile([P, NZP], F32, tag="nmf", name=uname("nmf"),
                                  bufs=NB + 1)
                    nc.vector.tensor_copy(ft[:], pt[:])
                    if f32_dram is not None:
                        nc.sync.dma_start(f32_dram[nb * P:(nb + 1) * P, :], ft[:])
                f32s.append(ft)
            return f32s

        def student_t(lhsT_aug, nm_f32s, out_dram):
            for nb in range(NB):
                pq = psum([P, NZP])
                nc.tensor.matmul(pq[:], lhsT_aug[:, nb * P:(nb + 1) * P],
                                 wqw[:, 0, :], start=True, stop=True)
                sqv = tmp.tile([P, NZ], F32, tag="q_sq", name=uname("q_sq"))
                nc.vector.tensor_mul(sqv[:], nm_f32s[nb][:, 0:NZ],
                                     nm_f32s[nb][:, 0:NZ])
                zn = tmp.tile([P, 1], F32, tag="q_zn", name=uname("q_zn"))
                nc.vector.tensor_reduce(zn[:], sqv[:], mybir.AxisListType.X, ALU.add)
                d2 = tmp.tile([P, NZ], F32, tag="q_d2", name=uname("q_d2"))
                nc.vector.tensor_scalar(d2[:], pq[:, 0:NZ], zn[:], None, ALU.add)
                qn = tmp.tile([P, NZ], F32, tag="q_qn", name=uname("q_qn"))
                nc.vector.reciprocal(qn[:], d2[:])
                sm = tmp.tile([P, 1], F32, tag="q_sm", name=uname("q_sm"))
                nc.vector.tensor_reduce(sm[:], qn[:], mybir.AxisListType.X, ALU.add)
                rs = tmp.tile([P, 1], F32, tag="q_rs", name=uname("q_rs"))
                nc.vector.reciprocal(rs[:], sm[:])
                ot = tmp.tile([P, NZP], F32, tag="q_ot", name=uname("q_ot"))
                nc.vector.memset(ot[:], 0.0)
                nc.vector.tensor_scalar(ot[:, 0:NZ], qn[:], rs[:], None, ALU.mult)
                nc.sync.dma_start(out_dram[nb * P:(nb + 1) * P, :], ot[:])

        # ================= phase A: t1 + enc1 (shared xT stream) ============
        t1_loc = tlocp.tile([P, NB, EP], BF, tag="tloc", name=uname("tloc"), bufs=2)
        r_e1T = pers.tile([P, E_CH, R], BF, tag="r_e1T")
        ps_t1 = [psum([P, EP]) for _ in range(NB)]
        ps_e1 = [psum([P, R]) for _ in range(E_CH)]
        for k_ in range(NIN_CH):
            xt = tst.tile([P, R], BF, tag="t", name=uname("xt"))
            nc.sync.dma_start(xt[:], xT_p.ap()[k_ * P:(k_ + 1) * P, :])
            g1t = wtile(wp["g1_w"], k_, 0, EP)
            e1t = wtile(wp["enc1_w"], k_, 0, EP)
            for nb in range(NB):
                nc.tensor.matmul(ps_t1[nb][:], xt[:, nb * P:(nb + 1) * P], g1t[:],
                                 start=(k_ == 0), stop=(k_ == NIN_CH - 1))
            for fo in range(E_CH):
                nc.tensor.matmul(ps_e1[fo][:], e1t[:, fo * P:(fo + 1) * P], xt[:],
                                 start=(k_ == 0), stop=(k_ == NIN_CH - 1))
        for nb in range(NB):
            nc.vector.tensor_copy(t1_loc[:, nb, :], ps_t1[nb][:])
        for fo in range(E_CH):
            nc.scalar.activation(r_e1T[:, fo, :], ps_e1[fo][:], AF.Relu,
                                 bias=bias["enc1_b"][:, fo:fo + 1])
        t1_full = allgather(t1_loc, EP, "t1")

        # AE encoder continues (overlaps t1 gather)
        r_e2T = pers.tile([P, E_CH, R], BF, tag="r_e2T")
        fmajor_linear(r_e2T, lambda k: r_e1T[:, k, :], wp["enc2_w"], E_CH, EP,
                      bias["enc2_b"])
        # r_e3 = relu(enc3.T @ r_e2 + b), fused with r = r_e3 @ zl_w + zl_b
        pr = psum([NZP, R])
        for g0 in range(0, E3_CH, 4):
            pss = [psum([P, R]) for _ in range(4)]
            for k_ in range(E_CH):
                wt = wtile(wp["enc3_w"], k_, g0 * P, 4 * P)
                for j in range(4):
                    nc.tensor.matmul(pss[j][:], wt[:, j * P:(j + 1) * P],
                                     r_e2T[:, k_, :], start=(k_ == 0),
                                     stop=(k_ == E_CH - 1))
            for j in range(4):
                fo = g0 + j
                d3t = sbst.tile([P, R], BF, tag="sb", name=uname("re3"))
                nc.scalar.activation(d3t[:], pss[j][:], AF.Relu,
                                     bias=bias["enc3_b"][:, fo:fo + 1])
                nc.tensor.matmul(pr[:], zlw[:, fo, :], d3t[:],
                                 start=(fo == 0), stop=(fo == E3_CH - 1))
        rT = pers.tile([P, R], BF, tag="rT")
        nc.gpsimd.memset(rT[:], 0.0)
        nc.scalar.activation(rT[0:NZP, :], pr[:], AF.Identity,
                             bias=bias["zl_b"][:, 0:1])
        nc.sync.dma_start(rT[NZ:NZ + 1, :], ones_row[0:1, 0:R])  # student-t aug
        r_nm_bf = pers.tile([P, NB, NZP], BF, tag="r_nm")
        r_f32s = narrow_nm(rT, bf_dst=r_nm_bf, f32_dram=o_small["r"].ap(),
                           want_f32=True)
        student_t(rT, r_f32s, o_small["q1"].ap())

        # ---- AE decoder (dec1 fused into dec2's k loop) ----
        r_d2T = pers.tile([P, E_CH, R], BF, tag="r_d2T")
        ps_d2 = [psum([P, R]) for _ in range(E_CH)]
        for k_ in range(NIN_CH):
            pd1 = psum([P, R])
            nc.tensor.matmul(pd1[:], dec1w[:, 0, k_ * P:(k_ + 1) * P], rT[:],
                             start=True, stop=True)
            d1t = tmp.tile([P, R], BF, tag="d1t", name=uname("d1t"))
            nc.scalar.activation(d1t[:], pd1[:], AF.Relu,
                                 bias=bias["dec1_b"][:, k_:k_ + 1])
            w2t = wtile(wp["dec2_w"], k_, 0, EP)
            for fo in range(E_CH):
                nc.tensor.matmul(ps_d2[fo][:], w2t[:, fo * P:(fo + 1) * P], d1t[:],
                                 start=(k_ == 0), stop=(k_ == NIN_CH - 1))
        for fo in range(E_CH):
            nc.scalar.activation(r_d2T[:, fo, :], ps_d2[fo][:], AF.Relu,
                                 bias=bias["dec2_b"][:, fo:fo + 1])
        r_d3T = pers.tile([P, E_CH, R], BF, tag="r_d3T")
        fmajor_linear(r_d3T, lambda k: r_d2T[:, k, :], wp["dec3_w"], E_CH, EP,
                      bias["dec3_b"])
        # x_bar (fp32 out, feature-major)
        for g0 in range(0, NIN_CH, 4):
            pss = [psum([P, R]) for _ in range(4)]
            for k_ in range(E_CH):
                wt = wtile(wp["xbar_w"], k_, g0 * P, 4 * P)
                for j in range(4):
                    nc.tensor.matmul(pss[j][:], wt[:, j * P:(j + 1) * P],
                                     r_d3T[:, k_, :], start=(k_ == 0),
                                     stop=(k_ == E_CH - 1))
            for j in range(4):
                fo = g0 + j
                xo = tmp.tile([P, R], F32, tag="fo32", name=uname("xo"), bufs=2)
                nc.scalar.activation(xo[:], pss[j][:], AF.Identity,
                                     bias=bias["xbar_b"][:, fo:fo + 1])
                nc.sync.dma_start(o_xbarT.ap()[fo * P:(fo + 1) * P, :], xo[:])

        # ================= GNN encoder =================
        z1T = pers.tile([P, E_CH, R], BF, tag="z1T")
        spmm_f(z1T, t1_full, EP, AF.Relu)
        p1 = gate([(lambda c: r_e1T[:, c, :], E_CH), (lambda c: z1T[:, c, :], E_CH)],
                  m1w, 8, bias["m1_b"], 2)
        t2_loc = tlocp.tile([P, NB, EP], BF, tag="tloc", name=uname("tloc"), bufs=2)
        nmajor_linear(t2_loc, lambda k: mix_chunk(z1T[:, k, :], r_e1T[:, k, :],
                                                  p1[0], p1[1]),
                      wp["g2_w"], E_CH, EP)
        t2_full = allgather(t2_loc, EP, "t2")

        z2T = pers.tile([P, E_CH, R], BF, tag="z2T")
        spmm_f(z2T, t2_full, EP, AF.Relu)
        t3_loc = tlocp.tile([P, NB, E3P], BF, tag="tlocbig", name=uname("tloc"))
        nmajor_linear(t3_loc, lambda k: z2T[:, k, :], wp["g3_w"], E_CH, E3P)
        t3_full = allgather(t3_loc, E3P, "t3")

        p2 = gate([(lambda c: r_e2T[:, c, :], E_CH), (lambda c: z2T[:, c, :], E_CH)],
                  m2w, 8, bias["m2_b"], 2)
        # h3 = relu(enc3.T @ mix2 + b) -> DRAM spill (2000-wide)
        h3_d = dram.tile([E3P, R], BF, tag="h3_d", name="h3_d")
        mix2T = pers.tile([P, E_CH, R], BF, tag="mix2T")
        for c in range(E_CH):
            mix_chunk(z2T[:, c, :], r_e2T[:, c, :], p2[0], p2[1],
                      out_ap=mix2T[:, c, :])
        fmajor_linear(None, lambda k: mix2T[:, k, :], wp["enc3_w"], E_CH, E3P,
                      bias["enc3_b"], dst_dram=h3_d[:])

        # z3 spmm -> DRAM spill
        z3_d = dram.tile([E3P, R], BF, tag="z3_d", name="z3_d")
        spmm_f(None, t3_full, E3P, AF.Relu, dst_dram=z3_d[:])

        p3 = gate([(lambda c: sb_chunk(h3_d[:], c), E3_CH),
                   (lambda c: sb_chunk(z3_d[:], c), E3_CH)],
                  m3w, 32, bias["m3_b"], 2)
        t4_loc = pers.tile([P, NB, NZP], BF, tag="t4_loc")
        nmajor_linear(t4_loc, lambda k: mix_chunk(sb_chunk(z3_d[:], k),
                                                  sb_chunk(h3_d[:], k),
                                                  p3[0], p3[1]),
                      wp["g4_w"], E3_CH, NZP, wtiles=g4w)
        t4_full = allgather(t4_loc, NZP, "t4")

        # z (narrow spmm, feature-major) + relu
        pz = spmm_narrow(t4_full)
        zT = pers.tile([P, R], BF, tag="zT")
        nc.gpsimd.memset(zT[:], 0.0)
        nc.scalar.activation(zT[0:NZP, :], pz[:], AF.Relu)
        # node-major z, z+r
        zr_loc = pers.tile([P, NB, NZP], BF, tag="zr_loc")
        for nb in range(NB):
            pt = psum([P, NZP], BF)
            nc.tensor.transpose(pt[:], zT[0:NZP, nb * P:(nb + 1) * P],
                                ident[0:NZP, 0:NZP])
            nc.vector.tensor_add(zr_loc[:, nb, :], pt[:], r_nm_bf[:, nb, :])
            zf = tmp.tile([P, NZP], F32, tag="nmf", name=uname("zf"), bufs=NB + 1)
            nc.vector.tensor_copy(zf[:], pt[:])
            nc.sync.dma_start(o_small["z"].ap()[nb * P:(nb + 1) * P, :], zf[:])
        zr_full = allgather(zr_loc, NZP, "zr")

        # t6 = z @ g6 (node-major [R, NINP])
        t6_loc = tlocp.tile([P, NB, NINP], BF, tag="tlocbig", name=uname("tloc"))
        for fog in range(NIN_CH // 4):
            pss = [psum([P, 512]) for _ in range(NB)]
            for nb in range(NB):
                nc.tensor.matmul(pss[nb][:], zT[:, nb * P:(nb + 1) * P],
                                 g6w[:, 0, fog * 512:(fog + 1) * 512],
                                 start=True, stop=True)
            for nb in range(NB):
                nc.vector.tensor_copy(t6_loc[:, nb, fog * 512:(fog + 1) * 512],
                                      pss[nb][:])
        t6_full = allgather(t6_loc, NINP, "t6")

        # z_l = adj @ (z + r)
        pzl = spmm_narrow(zr_full)
        z_lT = pers.tile([P, R], BF, tag="z_lT")
        nc.gpsimd.memset(z_lT[:], 0.0)
        nc.vector.tensor_copy(z_lT[0:NZP, :], pzl[:])
        nc.sync.dma_start(z_lT[NZ:NZ + 1, :], ones_row[0:1, 0:R])
        zl_f32s = narrow_nm(z_lT, f32_dram=o_small["zl"].ap(), want_f32=True)
        student_t(z_lT, zl_f32s, o_small["q"].ap())

        # ================= GNN decoder =================
        dz1_d = dram.tile([NINP, R], BF, tag="dz1_d", name="dz1_d")
        spmm_f(None, t6_full, NINP, AF.Relu, dst_dram=dz1_d[:])
        t7_loc = tlocp.tile([P, NB, EP], BF, tag="tloc", name=uname("tloc"), bufs=2)
        nmajor_linear(t7_loc, lambda k: sb_chunk(dz1_d[:], k), wp["g7_w"],
                      NIN_CH, EP)
        t7_full = allgather(t7_loc, EP, "t7")

        dz2T = pers.tile([P, E_CH, R], BF, tag="dz2T")
        spmm_f(dz2T, t7_full, EP, AF.Relu)
        t8_loc = tlocp.tile([P, NB, EP], BF, tag="tloc", name=uname("tloc"), bufs=2)
        nmajor_linear(t8_loc, lambda k: dz2T[:, k, :], wp["g8_w"], E_CH, EP)
        t8_full = allgather(t8_loc, EP, "t8")

        dz3T = pers.tile([P, E_CH, R], BF, tag="dz3T")
        spmm_f(dz3T, t8_full, EP, AF.Relu)
        t9_loc = tlocp.tile([P, NB, NINP], BF, tag="tlocbig", name=uname("tloc"))
        nmajor_linear(t9_loc, lambda k: dz3T[:, k, :], wp["g9_w"], E_CH, NINP)
        t9_full = allgather(t9_loc, NINP, "t9")

        # z_hat spmm: keep bf16 resident + write fp32 output
        z_hatT = pers.tile([P, NIN_CH, R], BF, tag="z_hatT")
        spmm_f(z_hatT, t9_full, NINP, AF.Relu, fp32_dram=o_zhatT.ap())

        # gather z_hatT (feature-major blocks) for adj_hat
        zh_bounce = dram.tile([NINP, R], BF, tag="b_zh", name="b_zh")
        nc.sync.dma_start(zh_bounce[:].rearrange("(po pi) f -> pi po f", pi=P),
                          z_hatT[:])
        zh_full = dram.tile([NCORES * NINP, R], BF, tag="g_zh", name="g_zh",
                            addr_space="Shared")
        nc.gpsimd.collective_compute("AllGather", ALU.bypass, replica_groups=rg,
                                     ins=[zh_bounce[:].opt()],
                                     outs=[zh_full[:].opt()])

        # ---- pred path: net = w_i * z_i concat, t_a = net @ agnn ----
        zT3 = zT.rearrange("p (c f) -> p c f", c=1)
        pml = gate([(lambda c: z1T[:, c, :], E_CH), (lambda c: z2T[:, c, :], E_CH),
                    (lambda c: sb_chunk(z3_d[:], c), E3_CH),
                    (lambda c: zT3[:, 0, :], 1)],
                   mlw, CAT_CH, bias["ml_b"], 4)
        ta_loc = pers.tile([P, NB, NZP], BF, tag="ta_loc")
        ps_ta = [psum([P, NZP]) for _ in range(NB)]
        blocks = [(lambda c: z1T[:, c, :], E_CH, pml[0]),
                  (lambda c: z2T[:, c, :], E_CH, pml[1]),
                  (lambda c: sb_chunk(z3_d[:], c), E3_CH, pml[2]),
                  (lambda c: zT3[:, 0, :], 1, pml[3])]
        k_ = 0
        for fn, nch, pb_ in blocks:
            for c in range(nch):
                mz = tmp.tile([P, R], BF, tag="mz", name=uname("mz"))
                nc.vector.tensor_mul(mz[:], fn(c), pb_[:])
                for nb in range(NB):
                    nc.tensor.matmul(ps_ta[nb][:], mz[:, nb * P:(nb + 1) * P],
                                     agnnw[:, k_, :], start=(k_ == 0),
                                     stop=(k_ == CAT_CH - 1))
                k_ += 1
        for nb in range(NB):
            nc.vector.tensor_copy(ta_loc[:, nb, :], ps_ta[nb][:])
        ta_full = allgather(ta_loc, NZP, "ta")

        # pred = softmax(adj @ t_a) node-major
        ps_pred = [psum([P, NZP]) for _ in range(NB)]
        for col in range(CC):
            tt = tst.tile([P, NZP], BF, tag="tn", name=uname("tn"))
            nc.sync.dma_start(tt[:], ta_full[col * P:(col + 1) * P, :])
            for nb in range(NB):
                nc.tensor.matmul(ps_pred[nb][:], adjT[:, col, nb * P:(nb + 1) * P],
                                 tt[:], start=(col == 0), stop=(col == CC - 1))
        for nb in range(NB):
            ex = tmp.tile([P, NZ], F32, tag="pr_e", name=uname("pr_e"))
            s = tmp.tile([P, 1], F32, tag="pr_s", name=uname("pr_s"))
            nc.scalar.activation(ex[:], ps_pred[nb][:, 0:NZ], AF.Exp, accum_out=s[:])
            ri = tmp.tile([P, 1], F32, tag="pr_ri", name=uname("pr_ri"))
            nc.vector.reciprocal(ri[:], s[:])
            po = tmp.tile([P, NZP], F32, tag="pr_o", name=uname("pr_o"))
            nc.vector.memset(po[:], 0.0)
            nc.vector.tensor_scalar(po[:, 0:NZ], ex[:], ri[:], None, ALU.mult)
            nc.sync.dma_start(o_small["pred"].ap()[nb * P:(nb + 1) * P, :], po[:])

        # ---- adj_hat = sigmoid(z_hat @ z_hat^T), row-blocks x col-blocks ----
        for cb in range(NCORES):
            pss = [psum([P, R]) for _ in range(NB)]
            for f in range(NIN_CH):
                rt = tst.tile([P, R], BF, tag="t", name=uname("rt"))
                nc.sync.dma_start(rt[:], zh_full[cb * NINP + f * P:
                                                 cb * NINP + (f + 1) * P, :])
                for nb in range(NB):
                    nc.tensor.matmul(pss[nb][:], z_hatT[:, f, nb * P:(nb + 1) * P],
                                     rt[:], start=(f == 0), stop=(f == NIN_CH - 1))
            for nb in range(NB):
                so = tmp.tile([P, R], F32, tag="fo32", name=uname("ah"), bufs=2)
                nc.scalar.activation(so[:], pss[nb][:], AF.Sigmoid)
                nc.sync.dma_start(o_adjhat.ap()[nb * P:(nb + 1) * P,
                                                cb * R:(cb + 1) * R], so[:])

    nc.compile()
    return nc


# ----------------------------------------------------------------------------
# host-side input prep
# ----------------------------------------------------------------------------

def _pad2(a, s0, s1):
    z = np.zeros((s0, s1), np.float32)
    z[:a.shape[0], :a.shape[1]] = a
    return z


def prep_in_maps(inputs, N=N_FULL):
    R = N // NCORES
    f32 = {k: np.asarray(v, np.float32) for k, v in inputs.items()}

    shared = {}
    for nm, s0, s1 in [
        ("enc1_w", NINP, EP), ("enc2_w", EP, EP), ("enc3_w", EP, E3P),
        ("zl_w", E3P, NZP), ("dec1_w", P, NINP), ("dec2_w", NINP, EP),
        ("dec3_w", EP, EP), ("xbar_w", EP, NINP),
        ("g1_w", NINP, EP), ("g2_w", EP, EP), ("g3_w", EP, E3P),
        ("g4_w", E3P, NZP), ("g6_w", P, NINP), ("g7_w", NINP, EP),
        ("g8_w", EP, EP), ("g9_w", EP, NINP),
    ]:
        shared[nm] = _pad2(f32[nm], s0, s1).astype(bf16)

    def blocks2(w, bsz, bpad):
        nb_ = w.shape[0] // bsz
        out = np.zeros((nb_ * bpad, w.shape[1]), np.float32)
        for i in range(nb_):
            out[i * bpad:i * bpad + bsz] = w[i * bsz:(i + 1) * bsz]
        return out

    shared["m1_w"] = blocks2(f32["m1_w"], E1, EP).astype(bf16)
    shared["m2_w"] = blocks2(f32["m2_w"], E1, EP).astype(bf16)
    shared["m3_w"] = blocks2(f32["m3_w"], E3, E3P).astype(bf16)

    def blocks_cat(w, ncols=None):
        out = np.zeros((CATP, ncols or w.shape[1]), np.float32)
        out[0:E1, :w.shape[1]] = w[0:E1]
        out[EP:EP + E1, :w.shape[1]] = w[E1:2 * E1]
        out[2 * EP:2 * EP + E3, :w.shape[1]] = w[2 * E1:2 * E1 + E3]
        out[2 * EP + E3P:2 * EP + E3P + NZ, :w.shape[1]] = w[2 * E1 + E3:2 * E1 + E3 + NZ]
        return out

    shared["ml_w"] = blocks_cat(f32["ml_w"]).astype(bf16)
    shared["agnn_w"] = blocks_cat(f32["agnn_w"], NZP).astype(bf16)

    cl = f32["cluster"]                      # [K, NZ]
    wq = np.zeros((P, NZP), np.float32)
    wq[0:NZ, 0:K] = -2.0 * cl.T
    wq[NZ, 0:K] = (cl * cl).sum(axis=1) + 1.0
    shared["wq"] = wq.astype(bf16)

    for nm, total, nch in [
        ("enc1_b", EP, E_CH), ("enc2_b", EP, E_CH), ("enc3_b", E3P, E3_CH),
        ("dec1_b", NINP, NIN_CH), ("dec2_b", EP, E_CH), ("dec3_b", EP, E_CH),
        ("xbar_b", NINP, NIN_CH),
    ]:
        b = np.zeros(total, np.float32)
        b[:f32[nm].shape[0]] = f32[nm]
        shared[nm] = np.ascontiguousarray(b.reshape(nch, P).T)
    shared["zl_b"] = _pad2(f32["zl_b"][:, None], NZP, 1)
    for nm, nl in [("m1_b", 2), ("m2_b", 2), ("m3_b", 2), ("ml_b", 4)]:
        shared[nm] = np.ascontiguousarray(f32[nm].reshape(nl, 1))

    adjT = np.ascontiguousarray(f32["adj"].T).astype(bf16)      # [N, N]
    xT = np.zeros((NINP, N), np.float32)
    xT[0:NIN] = f32["x"].T
    xT = xT.astype(bf16)

    in_maps = []
    for c in range(NCORES):
        m = dict(shared)
        m["adjT"] = np.ascontiguousarray(adjT[:, c * R:(c + 1) * R])
        m["xT"] = np.ascontiguousarray(xT[:, c * R:(c + 1) * R])
        in_maps.append(m)
    return in_maps


def assemble_outputs(results, N=N_FULL):
    def cat_rows(key):
        return np.concatenate([r[key] for r in results], axis=0)

    x_bar = np.concatenate([r["o_xbarT"][0:NIN, :].T for r in results], axis=0)
    z_hat = np.concatenate([r["o_zhatT"][0:NIN, :].T for r in results], axis=0)
    adj_hat = cat_rows("o_adjhat")
    q = cat_rows("o_q")[:, 0:NZ]
    q1 = cat_rows("o_q1")[:, 0:NZ]
    z = cat_rows("o_z")[:, 0:NZ]
    r_ = cat_rows("o_r")[:, 0:NZ]
    z_l = cat_rows("o_zl")[:, 0:NZ]
    pred = cat_rows("o_pred")[:, 0:NZ]
    return (x_bar, z_hat, adj_hat, q, q1, z, r_, z_l, pred)


def _run(inputs, trace=False):
    if N_FULL not in _BUILD_CACHE:
        _BUILD_CACHE[N_FULL] = build_graph(N_FULL)
    nc = _BUILD_CACHE[N_FULL]
    in_maps = prep_in_maps(inputs, N_FULL)
    res = run_bass_kernel_spmd(nc, in_maps, list(range(NCORES)), trace=trace)
    outs = assemble_outputs(res.results, N_FULL)
    return outs, res


def kernel(**inputs):
    outs, _ = _run(inputs, trace=False)
    return outs


# revision 18
# speedup vs baseline: 1.0912x; 1.0912x over previous
"""Trainium2 Bass kernel for nn_Adiin_24197845746021 (gnn_message_passing).

Row-shard the N=4096 nodes across 8 NeuronCores (512 rows each). Each core
holds adj[rows,:].T (bf16) resident in SBUF. adj @ X is computed as
lhsT = X_full tile (AllGathered, node-major), rhs = adjT shard ->
feature-major output. Feature matmuls consume feature-major activations
directly, so no transposes are needed except for the tiny [*,10] tensors.
All matmuls bf16 (fp32 PSUM); gating / softmax / student-t math fp32.
Contractions are zero-padded to multiples of 128 host-side. Wide (2000-dim)
mid-lived activations (z3, h3, dz1) spill to DRAM and are re-streamed.
"""

import numpy as np
import ml_dtypes

import concourse.bass as bass
import concourse.mybir as mybir
import concourse.tile as tile
from concourse import bacc
from concourse.bass_utils import run_bass_kernel_spmd
from concourse.masks import make_identity

BF = mybir.dt.bfloat16
F32 = mybir.dt.float32
AF = mybir.ActivationFunctionType
ALU = mybir.AluOpType
bf16 = ml_dtypes.bfloat16

P = 128
NCORES = 8
N_FULL = 4096
NIN, E1, E3, NZ, K = 2000, 500, 2000, 10, 10
NINP, EP, E3P, NZP = 2048, 512, 2048, 16       # padded dims
NIN_CH, E_CH, E3_CH = NINP // P, EP // P, E3P // P   # 16, 4, 16
CATP = EP + EP + E3P + P                        # 3200: [z1|z2|z3|z] padded
CAT_CH = CATP // P                              # 25

_BUILD_CACHE = {}


def build_graph(N=N_FULL):
    R = N // NCORES          # rows per core
    NB = R // P              # node blocks per core
    CC = N // P              # contraction (column) chunks for spmm

    nc = bacc.Bacc("TRN2", target_bir_lowering=False, debug=False,
                   num_devices=NCORES)

    def din(name, shape, dt=BF):
        return nc.declare_dram_parameter(name, list(shape), dt, isOutput=False)

    def dout(name, shape):
        return nc.declare_dram_parameter(name, list(shape), F32, isOutput=True)

    adjT_p = din("adjT", [N, R])
    xT_p = din("xT", [NINP, R])
    wp = {}
    for nm, sh in [
        ("enc1_w", (NINP, EP)), ("enc2_w", (EP, EP)), ("enc3_w", (EP, E3P)),
        ("zl_w", (E3P, NZP)), ("dec1_w", (P, NINP)), ("dec2_w", (NINP, EP)),
        ("dec3_w", (EP, EP)), ("xbar_w", (EP, NINP)),
        ("g1_w", (NINP, EP)), ("g2_w", (EP, EP)), ("g3_w", (EP, E3P)),
        ("g4_w", (E3P, NZP)), ("g6_w", (P, NINP)), ("g7_w", (NINP, EP)),
        ("g8_w", (EP, EP)), ("g9_w", (EP, NINP)),
        ("agnn_w", (CATP, NZP)), ("m1_w", (2 * EP, 2)), ("m2_w", (2 * EP, 2)),
        ("m3_w", (2 * E3P, 2)), ("ml_w", (CATP, 4)), ("wq", (P, NZP)),
    ]:
        wp[nm] = din(nm, sh)
    bp = {}
    for nm, sh in [
        ("enc1_b", (P, E_CH)), ("enc2_b", (P, E_CH)), ("enc3_b", (P, E3_CH)),
        ("zl_b", (NZP, 1)), ("dec1_b", (P, NIN_CH)), ("dec2_b", (P, E_CH)),
        ("dec3_b", (P, E_CH)), ("xbar_b", (P, NIN_CH)),
        ("m1_b", (2, 1)), ("m2_b", (2, 1)), ("m3_b", (2, 1)), ("ml_b", (4, 1)),
    ]:
        bp[nm] = din(nm, sh, F32)

    o_xbarT = dout("o_xbarT", [NINP, R])
    o_zhatT = dout("o_zhatT", [NINP, R])
    o_adjhat = dout("o_adjhat", [R, N])
    o_small = {nm: dout("o_" + nm, [R, NZP])
               for nm in ("q", "q1", "z", "r", "zl", "pred")}

    rg = [list(range(NCORES))]

    from contextlib import ExitStack
    with tile.TileContext(nc) as tc, ExitStack() as stack:
        pers = stack.enter_context(tc.tile_pool(name="pers", bufs=1))
        wst = stack.enter_context(tc.tile_pool(name="wst", bufs=3))
        tst = stack.enter_context(tc.tile_pool(name="tst", bufs=2))
        tmp = stack.enter_context(tc.tile_pool(name="tmp", bufs=2))
        sbst = stack.enter_context(tc.tile_pool(name="sbst", bufs=3))
        tlocp = stack.enter_context(tc.tile_pool(name="tlocp", bufs=1))
        ps = stack.enter_context(tc.tile_pool(name="ps", bufs=8, space="PSUM"))
        dram = stack.enter_context(tc.tile_pool(name="dram", bufs=1, space="DRAM"))

        _nmc = [0]

        def uname(pfx):
            _nmc[0] += 1
            return f"{pfx}{_nmc[0]}"

        # ---------------- constants / params in SBUF ----------------
        ident = pers.tile([P, P], BF, tag="ident")
        make_identity(nc, ident[:])
        ones_f = pers.tile([P, 1], F32, tag="ones_f")
        nc.gpsimd.memset(ones_f[:], 1.0)
        ones_row = pers.tile([1, 512], BF, tag="ones_row")
        nc.gpsimd.memset(ones_row[:], 1.0)

        adjT = pers.tile([P, CC, R], BF, tag="adjT")
        nc.sync.dma_start(adjT[:], adjT_p.ap().rearrange("(po pi) f -> pi po f", pi=P))

        bias = {}
        for nm in bp:
            t = pers.tile(list(bp[nm].shape), F32, tag="b_" + nm, name="b_" + nm)
            nc.sync.dma_start(t[:], bp[nm].ap())
            bias[nm] = t

        def resident_w(nm, nch, nl):
            t = pers.tile([P, nch, nl], BF, tag="w_" + nm, name="w_" + nm)
            nc.sync.dma_start(t[:], wp[nm].ap().rearrange("(po pi) l -> pi po l", pi=P))
            return t

        m1w = resident_w("m1_w", 8, 2)
        m2w = resident_w("m2_w", 8, 2)
        m3w = resident_w("m3_w", 32, 2)
        mlw = resident_w("ml_w", CAT_CH, 4)
        agnnw = resident_w("agnn_w", CAT_CH, NZP)
        zlw = resident_w("zl_w", E3_CH, NZP)
        g4w = resident_w("g4_w", E3_CH, NZP)
        wqw = resident_w("wq", 1, NZP)
        dec1w = resident_w("dec1_w", 1, NINP)
        g6w = resident_w("g6_w", 1, NINP)

        # ---------------- helpers ----------------
        def wtile(param, k, f0, fsz):
            t = wst.tile([P, fsz], BF, tag="w", name=uname("wt"))
            nc.sync.dma_start(t[:], param.ap()[k * P:(k + 1) * P, f0:f0 + fsz])
            return t

        def psum(shape, dt=F32):
            return ps.tile(list(shape), dt, tag="ps", name=uname("ps"))

        def sb_chunk(src_dram, c):
            """stream one feature-major [128, R] bf16 chunk back from DRAM"""
            t = sbst.tile([P, R], BF, tag="sb", name=uname("sb"))
            nc.sync.dma_start(t[:], src_dram[c * P:(c + 1) * P, :])
            return t[:]

        def fmajor_linear(dst, src_fn, wparam, nk, nfo_total, bias_t=None,
                          act=AF.Relu, dst_dram=None):
            """feature-major linear: out = act(W.T @ src + b).
            dst: [128, nfo, R] sbuf or None; dst_dram: [nfo*128, R] dram spill.
            src_fn(k) -> [128, R] AP."""
            nfo = nfo_total // P
            for g0 in range(0, nfo, 4):
                gn = min(4, nfo - g0)
                pss = [psum([P, R]) for _ in range(gn)]
                for k_ in range(nk):
                    wt = wtile(wparam, k_, g0 * P, gn * P)
                    src = src_fn(k_)
                    for j in range(gn):
                        nc.tensor.matmul(pss[j][:], wt[:, j * P:(j + 1) * P],
                                         src, start=(k_ == 0), stop=(k_ == nk - 1))
                for j in range(gn):
                    fo = g0 + j
                    b_ap = bias_t[:, fo:fo + 1] if bias_t is not None else 0.0
                    if dst is not None:
                        nc.scalar.activation(dst[:, fo, :], pss[j][:], act, bias=b_ap)
                    if dst_dram is not None:
                        ot = sbst.tile([P, R], BF, tag="sb", name=uname("fo"))
                        nc.scalar.activation(ot[:], pss[j][:], act, bias=b_ap)
                        nc.sync.dma_start(dst_dram[fo * P:(fo + 1) * P, :], ot[:])

        def nmajor_linear(dst, src_fn, wparam, nk, fo_total, wtiles=None,
                          after_fog=None):
            """node-major inner product: dst [128, NB, fo_total] = src.T @ W.
            src_fn(k) -> feature-major [128, R] chunk. after_fog: {fog: cb}."""
            for fog in range((fo_total + 511) // 512):
                fsz = min(512, fo_total - fog * 512)
                pss = [psum([P, fsz]) for _ in range(NB)]
                for k_ in range(nk):
                    if wtiles is not None:
                        wt = wtiles[:, k_, fog * 512:fog * 512 + fsz]
                    else:
                        wt = wtile(wparam, k_, fog * 512, fsz)[:]
                    src = src_fn(k_)
                    for nb in range(NB):
                        nc.tensor.matmul(pss[nb][:], src[:, nb * P:(nb + 1) * P],
                                         wt, start=(k_ == 0), stop=(k_ == nk - 1))
                for nb in range(NB):
                    nc.vector.tensor_copy(dst[:, nb, fog * 512:fog * 512 + fsz],
                                          pss[nb][:])
                if after_fog and fog in after_fog:
                    after_fog[fog]()

        def allgather(loc, w, name, col0=0, wsub=None):
            """loc [128, NB, w] bf16 node-major -> gathered [N, wsub] dram,
            gathering only columns [col0, col0+wsub)."""
            wsub = wsub or w
            bounce = dram.tile([R, wsub], BF, tag="b_" + name, name="b_" + name)
            nc.sync.dma_start(bounce[:].rearrange("(po pi) f -> pi po f", pi=P),
                              loc[:, :, col0:col0 + wsub])
            full = dram.tile([N, wsub], BF, tag="g_" + name, name="g_" + name,
                             addr_space="Shared")
            nc.gpsimd.collective_compute(
                "AllGather", ALU.bypass, replica_groups=rg,
                ins=[bounce[:].opt()], outs=[full[:].opt()])
            return full

        def spmm_pass(pss, tfull, pw):
            """one spmm accumulation pass; tfull [N, pw*128], pss: pw psums."""
            for c4 in range(CC // 4):
                tt = tst.tile([P, 4, pw * P], BF, tag="t", name=uname("t"))
                nc.sync.dma_start(
                    tt[:], tfull[c4 * 4 * P:(c4 + 1) * 4 * P, :]
                    .rearrange("(po pi) f -> pi po f", pi=P))
                for i4 in range(4):
                    col = c4 * 4 + i4
                    for j in range(pw):
                        nc.tensor.matmul(pss[j][:], tt[:, i4, j * P:(j + 1) * P],
                                         adjT[:, col, :], start=(col == 0),
                                         stop=(col == CC - 1))

        def spmm_f(dstT, fulls, w_total, act, dst_dram=None, fp32_dram=None,
                   epilogue=None):
            """feature-major spmm over pass-aligned gathered tensors.
            fulls: list of (dram_tensor, pass_width_chunks)."""
            fo = 0
            for tfull, pw in fulls:
                pss = [psum([P, R]) for _ in range(pw)]
                spmm_pass(pss, tfull, pw)
                for j in range(pw):
                    if fp32_dram is not None:
                        xo = tmp.tile([P, R], F32, tag="fo32", name=uname("fo32"),
                                      bufs=2)
                        nc.scalar.activation(xo[:], pss[j][:], act)
                        nc.sync.dma_start(fp32_dram[fo * P:(fo + 1) * P, :], xo[:])
                        if dstT is not None:
                            nc.vector.tensor_copy(dstT[:, fo, :], xo[:])
                    elif dstT is not None:
                        nc.scalar.activation(dstT[:, fo, :], pss[j][:], act)
                    elif dst_dram is not None:
                        ot = sbst.tile([P, R], BF, tag="sb", name=uname("so"))
                        nc.scalar.activation(ot[:], pss[j][:], act)
                        nc.sync.dma_start(dst_dram[fo * P:(fo + 1) * P, :], ot[:])
                    fo += 1
                if epilogue is not None:
                    epilogue(fo)

        def spmm_narrow(tfull):
            """narrow spmm: load whole [N, NZP] in one DMA, one psum out."""
            tt = tst.tile([P, CC, NZP], BF, tag="tn", name=uname("tn"))
            nc.sync.dma_start(tt[:],
                              tfull[:].rearrange("(po pi) f -> pi po f", pi=P))
            pp = psum([NZP, R])
            for col in range(CC):
                nc.tensor.matmul(pp[:], tt[:, col, :], adjT[:, col, :],
                                 start=(col == 0), stop=(col == CC - 1))
            return pp

        def gate(src_list, wres, nktot, bias_t, nl):
            """softmax(lrelu(cat @ w + b)) row-L2-normalized.
            src_list: [(chunk_fn, nch)]; returns nl fp32 [128,R] bcast tiles."""
            pp = psum([nl, R])
            k_ = 0
            for fn, nch in src_list:
                for c in range(nch):
                    nc.tensor.matmul(pp[:], wres[:, k_, :nl], fn(c),
                                     start=(k_ == 0), stop=(k_ == nktot - 1))
                    k_ += 1
            assert k_ == nktot
            e_full = tmp.tile([P, R], F32, tag="gt_e", name=uname("gt_e"), bufs=1)
            nc.vector.memset(e_full[:], 0.0)
            lin = tmp.tile([nl, R], F32, tag="gt_lin", name=uname("gt_lin"), bufs=1)
            nc.scalar.activation(lin[:], pp[:], AF.Identity, bias=bias_t[:, 0:1])
            sc = tmp.tile([nl, R], F32, tag="gt_sc", name=uname("gt_sc"), bufs=1)
            nc.vector.tensor_scalar_mul(sc[:], lin[:], 0.01)
            lr = tmp.tile([nl, R], F32, tag="gt_lr", name=uname("gt_lr"), bufs=1)
            nc.vector.tensor_max(lr[:], lin[:], sc[:])
            nc.scalar.activation(e_full[0:nl, :], lr[:], AF.Exp)
            s1 = psum([1, R])
            nc.tensor.matmul(s1[:], ones_f[:, 0:1], e_full[:], start=True, stop=True)
            rinv = tmp.tile([1, R], F32, tag="gt_rinv", name=uname("gt_rinv"), bufs=1)
            nc.vector.reciprocal(rinv[:], s1[:])
            rb = tmp.tile([P, R], F32, tag="gt_rb", name=uname("gt_rb"), bufs=1)
            nc.gpsimd.partition_broadcast(rb[:], rinv[:])
            p_t = tmp.tile([P, R], F32, tag="gt_p", name=uname("gt_p"), bufs=1)
            nc.vector.memset(p_t[:], 0.0)
            nc.vector.tensor_mul(p_t[0:nl, :], e_full[0:nl, :], rb[0:nl, :])
            sq = tmp.tile([P, R], F32, tag="gt_sq", name=uname("gt_sq"), bufs=1)
            nc.vector.memset(sq[:], 0.0)
            nc.vector.tensor_mul(sq[0:nl, :], p_t[0:nl, :], p_t[0:nl, :])
            n2 = psum([1, R])
            nc.tensor.matmul(n2[:], ones_f[:, 0:1], sq[:], start=True, stop=True)
            sqr = tmp.tile([1, R], F32, tag="gt_sqr", name=uname("gt_sqr"), bufs=1)
            nc.scalar.activation(sqr[:], n2[:], AF.Sqrt)
            rn = tmp.tile([1, R], F32, tag="gt_rn", name=uname("gt_rn"), bufs=1)
            nc.vector.reciprocal(rn[:], sqr[:])
            rnb = tmp.tile([P, R], F32, tag="gt_rnb", name=uname("gt_rnb"), bufs=1)
            nc.gpsimd.partition_broadcast(rnb[:], rn[:])
            nc.vector.tensor_mul(p_t[0:nl, :], p_t[0:nl, :], rnb[0:nl, :])
            outs = []
            for i in range(nl):
                row = tmp.tile([1, R], F32, tag=f"gt_row{i}", name=uname("gt_row"),
                               bufs=1)
                if i == 0:
                    nc.vector.tensor_copy(row[:], p_t[0:1, :])
                else:
                    nc.sync.dma_start(row[:], p_t[i:i + 1, :])
                pb = tmp.tile([P, R], F32, tag=f"gt_pb{i}", name=uname("gt_pb"),
                              bufs=1)
                nc.gpsimd.partition_broadcast(pb[:], row[:])
                outs.append(pb)
            return outs

        def mix_chunk(a_ap, b_ap, pa, pb_, out_ap=None):
            """one [128,R] bf16 chunk of pa*a + pb*b"""
            t0 = tmp.tile([P, R], F32, tag="mx0", name=uname("mx0"))
            t1 = tmp.tile([P, R], F32, tag="mx1", name=uname("mx1"))
            if out_ap is None:
                out_ap = tmp.tile([P, R], BF, tag="mxo", name=uname("mxo"))[:]
            nc.vector.tensor_mul(t0[:], a_ap, pa[:])
            nc.vector.tensor_mul(t1[:], b_ap, pb_[:])
            nc.vector.tensor_add(out_ap, t0[:], t1[:])
            return out_ap

        def narrow_nm(srcT, bf_dst=None, f32_dram=None, want_f32=False):
            f32s = []
            for nb in range(NB):
                pt = psum([P, NZP], BF)
                nc.tensor.transpose(pt[:], srcT[0:NZP, nb * P:(nb + 1) * P],
                                    ident[0:NZP, 0:NZP])
                if bf_dst is not None:
                    nc.vector.tensor_copy(bf_dst[:, nb, :], pt[:])
                ft = None
                if want_f32 or f32_dram is not None:
                    ft = tmp.tile([P, NZP], F32, tag="nmf", name=uname("nmf"),
                                  bufs=NB + 1)
                    nc.vector.tensor_copy(ft[:], pt[:])
                    if f32_dram is not None:
                        nc.sync.dma_start(f32_dram[nb * P:(nb + 1) * P, :], ft[:])
                f32s.append(ft)
            return f32s

        def student_t(lhsT_aug, nm_f32s, out_dram):
            for nb in range(NB):
                pq = psum([P, NZP])
                nc.tensor.matmul(pq[:], lhsT_aug[:, nb * P:(nb + 1) * P],
                                 wqw[:, 0, :], start=True, stop=True)
                sqv = tmp.tile([P, NZ], F32, tag="q_sq", name=uname("q_sq"))
                nc.vector.tensor_mul(sqv[:], nm_f32s[nb][:, 0:NZ],
                                     nm_f32s[nb][:, 0:NZ])
                zn = tmp.tile([P, 1], F32, tag="q_zn", name=uname("q_zn"))
                nc.vector.tensor_reduce(zn[:], sqv[:], mybir.AxisListType.X, ALU.add)
                d2 = tmp.tile([P, NZ], F32, tag="q_d2", name=uname("q_d2"))
                nc.vector.tensor_scalar(d2[:], pq[:, 0:NZ], zn[:], None, ALU.add)
                qn = tmp.tile([P, NZ], F32, tag="q_qn", name=uname("q_qn"))
                nc.vector.reciprocal(qn[:], d2[:])
                sm = tmp.tile([P, 1], F32, tag="q_sm", name=uname("q_sm"))
                nc.vector.tensor_reduce(sm[:], qn[:], mybir.AxisListType.X, ALU.add)
                rs = tmp.tile([P, 1], F32, tag="q_rs", name=uname("q_rs"))
                nc.vector.reciprocal(rs[:], sm[:])
                ot = tmp.tile([P, NZP], F32, tag="q_ot", name=uname("q_ot"))
                nc.vector.memset(ot[:], 0.0)
                nc.vector.tensor_scalar(ot[:, 0:NZ], qn[:], rs[:], None, ALU.mult)
                nc.sync.dma_start(out_dram[nb * P:(nb + 1) * P, :], ot[:])

        # ====================================================================
        # Schedule: the GNN chain is serial through 11 AllGathers; all
        # independent work (AE encoder/decoder, xbar, gates, student-t) is
        # placed to fill specific gather-latency windows.
        # ====================================================================

        # ---- phase A: t1 + enc1 share the streamed xT ----
        t1_loc = tlocp.tile([P, NB, EP], BF, tag="tloc", name=uname("tloc"), bufs=2)
        r_e1T = pers.tile([P, E_CH, R], BF, tag="r_e1T")
        ps_t1 = [psum([P, EP]) for _ in range(NB)]
        ps_e1 = [psum([P, R]) for _ in range(E_CH)]
        for k4 in range(NIN_CH // 4):
            xt4 = tst.tile([P, 4, R], BF, tag="t", name=uname("xt"))
            nc.sync.dma_start(xt4[:], xT_p.ap()[k4 * 4 * P:(k4 + 1) * 4 * P, :]
                              .rearrange("(po pi) f -> pi po f", pi=P))
            for i4 in range(4):
                k_ = k4 * 4 + i4
                xt = xt4[:, i4, :]
                g1t = wtile(wp["g1_w"], k_, 0, EP)
                e1t = wtile(wp["enc1_w"], k_, 0, EP)
                for nb in range(NB):
                    nc.tensor.matmul(ps_t1[nb][:], xt[:, nb * P:(nb + 1) * P],
                                     g1t[:], start=(k_ == 0),
                                     stop=(k_ == NIN_CH - 1))
                for fo in range(E_CH):
                    nc.tensor.matmul(ps_e1[fo][:], e1t[:, fo * P:(fo + 1) * P],
                                     xt, start=(k_ == 0), stop=(k_ == NIN_CH - 1))
        for nb in range(NB):
            nc.vector.tensor_copy(t1_loc[:, nb, :], ps_t1[nb][:])
        for fo in range(E_CH):
            nc.scalar.activation(r_e1T[:, fo, :], ps_e1[fo][:], AF.Relu,
                                 bias=bias["enc1_b"][:, fo:fo + 1])
        t1_full = allgather(t1_loc, EP, "t1")

        # [AG-t1 window] enc2
        r_e2T = pers.tile([P, E_CH, R], BF, tag="r_e2T")
        fmajor_linear(r_e2T, lambda k: r_e1T[:, k, :], wp["enc2_w"], E_CH, EP,
                      bias["enc2_b"])

        # ---- z1 ----
        z1T = pers.tile([P, E_CH, R], BF, tag="z1T")
        spmm_f(z1T, [(t1_full, 4)], EP, AF.Relu)
        p1 = gate([(lambda c: r_e1T[:, c, :], E_CH), (lambda c: z1T[:, c, :], E_CH)],
                  m1w, 8, bias["m1_b"], 2)
        t2_loc = tlocp.tile([P, NB, EP], BF, tag="tloc", name=uname("tloc"), bufs=2)
        nmajor_linear(t2_loc, lambda k: mix_chunk(z1T[:, k, :], r_e1T[:, k, :],
                                                  p1[0], p1[1]),
                      wp["g2_w"], E_CH, EP)
        t2_full = allgather(t2_loc, EP, "t2")

        # [AG-t2 window] r_e3 (fused with r = r_e3 @ zl_w + zl_b), then q1/r
        pr = psum([NZP, R])
        for g0 in range(0, E3_CH, 4):
            pss = [psum([P, R]) for _ in range(4)]
            for k_ in range(E_CH):
                wt = wtile(wp["enc3_w"], k_, g0 * P, 4 * P)
                for j in range(4):
                    nc.tensor.matmul(pss[j][:], wt[:, j * P:(j + 1) * P],
                                     r_e2T[:, k_, :], start=(k_ == 0),
                                     stop=(k_ == E_CH - 1))
            for j in range(4):
                fo = g0 + j
                d3t = sbst.tile([P, R], BF, tag="sb", name=uname("re3"))
                nc.scalar.activation(d3t[:], pss[j][:], AF.Relu,
                                     bias=bias["enc3_b"][:, fo:fo + 1])
                nc.tensor.matmul(pr[:], zlw[:, fo, :], d3t[:],
                                 start=(fo == 0), stop=(fo == E3_CH - 1))
        rT = pers.tile([P, R], BF, tag="rT")
        nc.gpsimd.memset(rT[:], 0.0)
        nc.scalar.activation(rT[0:NZP, :], pr[:], AF.Identity,
                             bias=bias["zl_b"][:, 0:1])
        nc.sync.dma_start(rT[NZ:NZ + 1, :], ones_row[0:1, 0:R])  # student-t aug
        r_nm_bf = pers.tile([P, NB, NZP], BF, tag="r_nm")
        r_f32s = narrow_nm(rT, bf_dst=r_nm_bf, f32_dram=o_small["r"].ap(),
                           want_f32=True)
        student_t(rT, r_f32s, o_small["q1"].ap())

        # ---- z2 ----
        z2T = pers.tile([P, E_CH, R], BF, tag="z2T")
        spmm_f(z2T, [(t2_full, 4)], EP, AF.Relu)
        # t3 = z2 @ g3 with half gathers
        t3_loc = tlocp.tile([P, NB, E3P], BF, tag="tlocbig", name=uname("tloc"))
        t3_h = []
        nmajor_linear(t3_loc, lambda k: z2T[:, k, :], wp["g3_w"], E_CH, E3P,
                      after_fog={1: lambda: t3_h.append(
                                     allgather(t3_loc, E3P, "t3a", 0, 1024)),
                                 3: lambda: t3_h.append(
                                     allgather(t3_loc, E3P, "t3b", 1024, 1024))})

        # [AG-t3 window] m2 gate, mix2, h3 -> DRAM
        p2 = gate([(lambda c: r_e2T[:, c, :], E_CH), (lambda c: z2T[:, c, :], E_CH)],
                  m2w, 8, bias["m2_b"], 2)
        h3_d = dram.tile([E3P, R], BF, tag="h3_d", name="h3_d")
        mix2T = pers.tile([P, E_CH, R], BF, tag="mix2T")
        for c in range(E_CH):
            mix_chunk(z2T[:, c, :], r_e2T[:, c, :], p2[0], p2[1],
                      out_ap=mix2T[:, c, :])
        fmajor_linear(None, lambda k: mix2T[:, k, :], wp["enc3_w"], E_CH, E3P,
                      bias["enc3_b"], dst_dram=h3_d[:])

        # ---- z3 (to DRAM) ----
        z3_d = dram.tile([E3P, R], BF, tag="z3_d", name="z3_d")
        spmm_f(None, [(t3_h[0], 8), (t3_h[1], 8)], E3P, AF.Relu,
               dst_dram=z3_d[:])

        # ---- m3 gate, t4 = mix3 @ g4 ----
        p3 = gate([(lambda c: sb_chunk(h3_d[:], c), E3_CH),
                   (lambda c: sb_chunk(z3_d[:], c), E3_CH)],
                  m3w, 32, bias["m3_b"], 2)
        t4_loc = pers.tile([P, NB, NZP], BF, tag="t4_loc")
        nmajor_linear(t4_loc, lambda k: mix_chunk(sb_chunk(z3_d[:], k),
                                                  sb_chunk(h3_d[:], k),
                                                  p3[0], p3[1]),
                      wp["g4_w"], E3_CH, NZP, wtiles=g4w)
        t4_full = allgather(t4_loc, NZP, "t4")

        # [AG-t4 window] dec1+dec2 fused
        r_d2T = pers.tile([P, E_CH, R], BF, tag="r_d2T")
        ps_d2 = [psum([P, R]) for _ in range(E_CH)]
        for k_ in range(NIN_CH):
            pd1 = psum([P, R])
            nc.tensor.matmul(pd1[:], dec1w[:, 0, k_ * P:(k_ + 1) * P], rT[:],
                             start=True, stop=True)
            d1t = tmp.tile([P, R], BF, tag="d1t", name=uname("d1t"))
            nc.scalar.activation(d1t[:], pd1[:], AF.Relu,
                                 bias=bias["dec1_b"][:, k_:k_ + 1])
            w2t = wtile(wp["dec2_w"], k_, 0, EP)
            for fo in range(E_CH):
                nc.tensor.matmul(ps_d2[fo][:], w2t[:, fo * P:(fo + 1) * P], d1t[:],
                                 start=(k_ == 0), stop=(k_ == NIN_CH - 1))
        for fo in range(E_CH):
            nc.scalar.activation(r_d2T[:, fo, :], ps_d2[fo][:], AF.Relu,
                                 bias=bias["dec2_b"][:, fo:fo + 1])

        # ---- z = relu(adj @ t4), node-major z, z+r ----
        pz = spmm_narrow(t4_full)
        zT = pers.tile([P, R], BF, tag="zT")
        nc.gpsimd.memset(zT[:], 0.0)
        nc.scalar.activation(zT[0:NZP, :], pz[:], AF.Relu)
        zr_loc = pers.tile([P, NB, NZP], BF, tag="zr_loc")
        for nb in range(NB):
            pt = psum([P, NZP], BF)
            nc.tensor.transpose(pt[:], zT[0:NZP, nb * P:(nb + 1) * P],
                                ident[0:NZP, 0:NZP])
            nc.vector.tensor_add(zr_loc[:, nb, :], pt[:], r_nm_bf[:, nb, :])
            zf = tmp.tile([P, NZP], F32, tag="nmf", name=uname("zf"), bufs=NB + 1)
            nc.vector.tensor_copy(zf[:], pt[:])
            nc.sync.dma_start(o_small["z"].ap()[nb * P:(nb + 1) * P, :], zf[:])
        zr_full = allgather(zr_loc, NZP, "zr")

        # [zr window] dec3
        r_d3T = pers.tile([P, E_CH, R], BF, tag="r_d3T")
        fmajor_linear(r_d3T, lambda k: r_d2T[:, k, :], wp["dec3_w"], E_CH, EP,
                      bias["dec3_b"])

        # ---- t6 = z @ g6, halves gathered ----
        t6_loc = tlocp.tile([P, NB, NINP], BF, tag="tlocbig", name=uname("tloc"))
        t6_h = []
        for fog in range(NIN_CH // 4):
            pss = [psum([P, 512]) for _ in range(NB)]
            for nb in range(NB):
                nc.tensor.matmul(pss[nb][:], zT[:, nb * P:(nb + 1) * P],
                                 g6w[:, 0, fog * 512:(fog + 1) * 512],
                                 start=True, stop=True)
            for nb in range(NB):
                nc.vector.tensor_copy(t6_loc[:, nb, fog * 512:(fog + 1) * 512],
                                      pss[nb][:])
            if fog == 1:
                t6_h.append(allgather(t6_loc, NINP, "t6a", 0, 1024))
            elif fog == 3:
                t6_h.append(allgather(t6_loc, NINP, "t6b", 1024, 1024))

        # [AG-t6 window] xbar groups 0,1 + z_l + q
        def xbar_group(g0):
            pss = [psum([P, R]) for _ in range(4)]
            for k_ in range(E_CH):
                wt = wtile(wp["xbar_w"], k_, g0 * P, 4 * P)
                for j in range(4):
                    nc.tensor.matmul(pss[j][:], wt[:, j * P:(j + 1) * P],
                                     r_d3T[:, k_, :], start=(k_ == 0),
                                     stop=(k_ == E_CH - 1))
            for j in range(4):
                fo = g0 + j
                xo = tmp.tile([P, R], F32, tag="fo32", name=uname("xo"), bufs=2)
                nc.scalar.activation(xo[:], pss[j][:], AF.Identity,
                                     bias=bias["xbar_b"][:, fo:fo + 1])
                nc.sync.dma_start(o_xbarT.ap()[fo * P:(fo + 1) * P, :], xo[:])

        xbar_group(0)
        pzl = spmm_narrow(zr_full)
        z_lT = pers.tile([P, R], BF, tag="z_lT")
        nc.gpsimd.memset(z_lT[:], 0.0)
        nc.vector.tensor_copy(z_lT[0:NZP, :], pzl[:])
        nc.sync.dma_start(z_lT[NZ:NZ + 1, :], ones_row[0:1, 0:R])
        zl_f32s = narrow_nm(z_lT, f32_dram=o_small["zl"].ap(), want_f32=True)
        student_t(z_lT, zl_f32s, o_small["q"].ap())
        xbar_group(4)

        # ---- dz1 (to DRAM) ----
        dz1_d = dram.tile([NINP, R], BF, tag="dz1_d", name="dz1_d")
        spmm_f(None, [(t6_h[0], 8), (t6_h[1], 8)], NINP, AF.Relu,
               dst_dram=dz1_d[:])
        t7_loc = tlocp.tile([P, NB, EP], BF, tag="tloc", name=uname("tloc"), bufs=2)
        nmajor_linear(t7_loc, lambda k: sb_chunk(dz1_d[:], k), wp["g7_w"],
                      NIN_CH, EP)
        t7_full = allgather(t7_loc, EP, "t7")

        # [AG-t7 window] ml gate + t_a + its gather, xbar group 2
        zT3 = zT.rearrange("p (c f) -> p c f", c=1)
        pml = gate([(lambda c: z1T[:, c, :], E_CH), (lambda c: z2T[:, c, :], E_CH),
                    (lambda c: sb_chunk(z3_d[:], c), E3_CH),
                    (lambda c: zT3[:, 0, :], 1)],
                   mlw, CAT_CH, bias["ml_b"], 4)
        ta_loc = pers.tile([P, NB, NZP], BF, tag="ta_loc")
        ps_ta = [psum([P, NZP]) for _ in range(NB)]
        blocks = [(lambda c: z1T[:, c, :], E_CH, pml[0]),
                  (lambda c: z2T[:, c, :], E_CH, pml[1]),
                  (lambda c: sb_chunk(z3_d[:], c), E3_CH, pml[2]),
                  (lambda c: zT3[:, 0, :], 1, pml[3])]
        k_ = 0
        for fn, nch, pb_ in blocks:
            for c in range(nch):
                mz = tmp.tile([P, R], BF, tag="mz", name=uname("mz"))
                nc.vector.tensor_mul(mz[:], fn(c), pb_[:])
                for nb in range(NB):
                    nc.tensor.matmul(ps_ta[nb][:], mz[:, nb * P:(nb + 1) * P],
                                     agnnw[:, k_, :], start=(k_ == 0),
                                     stop=(k_ == CAT_CH - 1))
                k_ += 1
        for nb in range(NB):
            nc.vector.tensor_copy(ta_loc[:, nb, :], ps_ta[nb][:])
        ta_full = allgather(ta_loc, NZP, "ta")
        xbar_group(8)

        # ---- dz2 ----
        dz2T = pers.tile([P, E_CH, R], BF, tag="dz2T")
        spmm_f(dz2T, [(t7_full, 4)], EP, AF.Relu)
        t8_loc = tlocp.tile([P, NB, EP], BF, tag="tloc", name=uname("tloc"), bufs=2)
        nmajor_linear(t8_loc, lambda k: dz2T[:, k, :], wp["g8_w"], E_CH, EP)
        t8_full = allgather(t8_loc, EP, "t8")

        # [AG-t8 window] pred = softmax(adj @ t_a), xbar group 3
        tta = tst.tile([P, CC, NZP], BF, tag="tn", name=uname("tta"))
        nc.sync.dma_start(tta[:],
                          ta_full[:].rearrange("(po pi) f -> pi po f", pi=P))
        ps_pred = [psum([P, NZP]) for _ in range(NB)]
        for col in range(CC):
            for nb in range(NB):
                nc.tensor.matmul(ps_pred[nb][:], adjT[:, col, nb * P:(nb + 1) * P],
                                 tta[:, col, :], start=(col == 0),
                                 stop=(col == CC - 1))
        for nb in range(NB):
            ex = tmp.tile([P, NZ], F32, tag="pr_e", name=uname("pr_e"))
            s = tmp.tile([P, 1], F32, tag="pr_s", name=uname("pr_s"))
            nc.scalar.activation(ex[:], ps_pred[nb][:, 0:NZ], AF.Exp, accum_out=s[:])
            ri = tmp.tile([P, 1], F32, tag="pr_ri", name=uname("pr_ri"))
            nc.vector.reciprocal(ri[:], s[:])
            po = tmp.tile([P, NZP], F32, tag="pr_o", name=uname("pr_o"))
            nc.vector.memset(po[:], 0.0)
            nc.vector.tensor_scalar(po[:, 0:NZ], ex[:], ri[:], None, ALU.mult)
            nc.sync.dma_start(o_small["pred"].ap()[nb * P:(nb + 1) * P, :], po[:])
        xbar_group(12)

        # ---- dz3 ----
        dz3T = pers.tile([P, E_CH, R], BF, tag="dz3T")
        spmm_f(dz3T, [(t8_full, 4)], EP, AF.Relu)
        t9_loc = tlocp.tile([P, NB, NINP], BF, tag="tlocbig", name=uname("tloc"))
        t9_h = []
        nmajor_linear(t9_loc, lambda k: dz3T[:, k, :], wp["g9_w"], E_CH, NINP,
                      after_fog={1: lambda: t9_h.append(
                                     allgather(t9_loc, NINP, "t9a", 0, 1024)),
                                 3: lambda: t9_h.append(
                                     allgather(t9_loc, NINP, "t9b", 1024, 1024))})

        # ---- z_hat: spmm halves, each followed by its zh gather half ----
        z_hatT = pers.tile([P, NIN_CH, R], BF, tag="z_hatT")
        zh_bounce = dram.tile([NINP, R], BF, tag="b_zh", name="b_zh")
        zh_h = []
        for p_i in range(2):
            pss = [psum([P, R]) for _ in range(8)]
            spmm_pass(pss, t9_h[p_i], 8)
            for j in range(8):
                fo = p_i * 8 + j
                xo = tmp.tile([P, R], F32, tag="fo32", name=uname("fo32"), bufs=2)
                nc.scalar.activation(xo[:], pss[j][:], AF.Relu)
                nc.sync.dma_start(o_zhatT.ap()[fo * P:(fo + 1) * P, :], xo[:])
                nc.vector.tensor_copy(z_hatT[:, fo, :], xo[:])
                nc.sync.dma_start(zh_bounce[fo * P:(fo + 1) * P, :],
                                  z_hatT[:, fo, :])
            full = dram.tile([NCORES * 1024, R], BF, tag=f"g_zh{p_i}",
                             name=f"g_zh{p_i}", addr_space="Shared")
            nc.gpsimd.collective_compute(
                "AllGather", ALU.bypass, replica_groups=rg,
                ins=[zh_bounce[p_i * 1024:(p_i + 1) * 1024, :].opt()],
                outs=[full[:].opt()])
            zh_h.append(full)

        # ---- adj_hat = sigmoid(z_hat @ z_hat^T) ----
        for cb in range(NCORES):
            pss = [psum([P, R]) for _ in range(NB)]
            for f4 in range(NIN_CH // 4):
                h = f4 // 2
                base = cb * 1024 + (f4 % 2) * 512
                rt4 = tst.tile([P, 4, R], BF, tag="t", name=uname("rt"))
                nc.sync.dma_start(rt4[:], zh_h[h][base:base + 512, :]
                                  .rearrange("(po pi) f -> pi po f", pi=P))
                for i4 in range(4):
                    f = f4 * 4 + i4
                    for nb in range(NB):
                        nc.tensor.matmul(pss[nb][:],
                                         z_hatT[:, f, nb * P:(nb + 1) * P],
                                         rt4[:, i4, :], start=(f == 0),
                                         stop=(f == NIN_CH - 1))
            for nb in range(NB):
                so = tmp.tile([P, R], F32, tag="fo32", name=uname("ah"), bufs=2)
                nc.scalar.activation(so[:], pss[nb][:], AF.Sigmoid)
                nc.sync.dma_start(o_adjhat.ap()[nb * P:(nb + 1) * P,
                                                cb * R:(cb + 1) * R], so[:])

    nc.compile()
    return nc


# ----------------------------------------------------------------------------
# host-side input prep
# ----------------------------------------------------------------------------

def _pad2(a, s0, s1):
    z = np.zeros((s0, s1), np.float32)
    z[:a.shape[0], :a.shape[1]] = a
    return z


def prep_in_maps(inputs, N=N_FULL):
    R = N // NCORES
    f32 = {k: np.asarray(v, np.float32) for k, v in inputs.items()}

    shared = {}
    for nm, s0, s1 in [
        ("enc1_w", NINP, EP), ("enc2_w", EP, EP), ("enc3_w", EP, E3P),
        ("zl_w", E3P, NZP), ("dec1_w", P, NINP), ("dec2_w", NINP, EP),
        ("dec3_w", EP, EP), ("xbar_w", EP, NINP),
        ("g1_w", NINP, EP), ("g2_w", EP, EP), ("g3_w", EP, E3P),
        ("g4_w", E3P, NZP), ("g6_w", P, NINP), ("g7_w", NINP, EP),
        ("g8_w", EP, EP), ("g9_w", EP, NINP),
    ]:
        shared[nm] = _pad2(f32[nm], s0, s1).astype(bf16)

    def blocks2(w, bsz, bpad):
        nb_ = w.shape[0] // bsz
        out = np.zeros((nb_ * bpad, w.shape[1]), np.float32)
        for i in range(nb_):
            out[i * bpad:i * bpad + bsz] = w[i * bsz:(i + 1) * bsz]
        return out

    shared["m1_w"] = blocks2(f32["m1_w"], E1, EP).astype(bf16)
    shared["m2_w"] = blocks2(f32["m2_w"], E1, EP).astype(bf16)
    shared["m3_w"] = blocks2(f32["m3_w"], E3, E3P).astype(bf16)

    def blocks_cat(w, ncols=None):
        out = np.zeros((CATP, ncols or w.shape[1]), np.float32)
        out[0:E1, :w.shape[1]] = w[0:E1]
        out[EP:EP + E1, :w.shape[1]] = w[E1:2 * E1]
        out[2 * EP:2 * EP + E3, :w.shape[1]] = w[2 * E1:2 * E1 + E3]
        out[2 * EP + E3P:2 * EP + E3P + NZ, :w.shape[1]] = w[2 * E1 + E3:2 * E1 + E3 + NZ]
        return out

    shared["ml_w"] = blocks_cat(f32["ml_w"]).astype(bf16)
    shared["agnn_w"] = blocks_cat(f32["agnn_w"], NZP).astype(bf16)

    cl = f32["cluster"]                      # [K, NZ]
    wq = np.zeros((P, NZP), np.float32)
    wq[0:NZ, 0:K] = -2.0 * cl.T
    wq[NZ, 0:K] = (cl * cl).sum(axis=1) + 1.0
    shared["wq"] = wq.astype(bf16)

    for nm, total, nch in [
        ("enc1_b", EP, E_CH), ("enc2_b", EP, E_CH), ("enc3_b", E3P, E3_CH),
        ("dec1_b", NINP, NIN_CH), ("dec2_b", EP, E_CH), ("dec3_b", EP, E_CH),
        ("xbar_b", NINP, NIN_CH),
    ]:
        b = np.zeros(total, np.float32)
        b[:f32[nm].shape[0]] = f32[nm]
        shared[nm] = np.ascontiguousarray(b.reshape(nch, P).T)
    shared["zl_b"] = _pad2(f32["zl_b"][:, None], NZP, 1)
    for nm, nl in [("m1_b", 2), ("m2_b", 2), ("m3_b", 2), ("ml_b", 4)]:
        shared[nm] = np.ascontiguousarray(f32[nm].reshape(nl, 1))

    adjT = np.ascontiguousarray(f32["adj"].T).astype(bf16)      # [N, N]
    xT = np.zeros((NINP, N), np.float32)
    xT[0:NIN] = f32["x"].T
    xT = xT.astype(bf16)

    in_maps = []
    for c in range(NCORES):
        m = dict(shared)
        m["adjT"] = np.ascontiguousarray(adjT[:, c * R:(c + 1) * R])
        m["xT"] = np.ascontiguousarray(xT[:, c * R:(c + 1) * R])
        in_maps.append(m)
    return in_maps


def assemble_outputs(results, N=N_FULL):
    def cat_rows(key):
        return np.concatenate([r[key] for r in results], axis=0)

    x_bar = np.concatenate([r["o_xbarT"][0:NIN, :].T for r in results], axis=0)
    z_hat = np.concatenate([r["o_zhatT"][0:NIN, :].T for r in results], axis=0)
    adj_hat = cat_rows("o_adjhat")
    q = cat_rows("o_q")[:, 0:NZ]
    q1 = cat_rows("o_q1")[:, 0:NZ]
    z = cat_rows("o_z")[:, 0:NZ]
    r_ = cat_rows("o_r")[:, 0:NZ]
    z_l = cat_rows("o_zl")[:, 0:NZ]
    pred = cat_rows("o_pred")[:, 0:NZ]
    return (x_bar, z_hat, adj_hat, q, q1, z, r_, z_l, pred)


def _run(inputs, trace=False):
    if N_FULL not in _BUILD_CACHE:
        _BUILD_CACHE[N_FULL] = build_graph(N_FULL)
    nc = _BUILD_CACHE[N_FULL]
    in_maps = prep_in_maps(inputs, N_FULL)
    res = run_bass_kernel_spmd(nc, in_maps, list(range(NCORES)), trace=trace)
    outs = assemble_outputs(res.results, N_FULL)
    return outs, res


def kernel(**inputs):
    outs, _ = _run(inputs, trace=False)
    return outs


# revision 20
# speedup vs baseline: 1.1332x; 1.0385x over previous
"""Trainium2 Bass kernel for nn_Adiin_24197845746021 (gnn_message_passing).

Row-shard the N=4096 nodes across 8 NeuronCores (512 rows each). Each core
holds adj[rows,:].T (bf16) resident in SBUF. adj @ X is computed as
lhsT = X_full tile (AllGathered, node-major), rhs = adjT shard ->
feature-major output. Feature matmuls consume feature-major activations
directly, so no transposes are needed except for the tiny [*,10] tensors.
All matmuls bf16 (fp32 PSUM); gating / softmax / student-t math fp32.
Contractions are zero-padded to multiples of 128 host-side. Wide (2000-dim)
mid-lived activations (z3, h3, dz1) spill to DRAM and are re-streamed.
"""

import numpy as np
import ml_dtypes

import concourse.bass as bass
import concourse.mybir as mybir
import concourse.tile as tile
from concourse import bacc
from concourse.bass_utils import run_bass_kernel_spmd
from concourse.masks import make_identity

BF = mybir.dt.bfloat16
F32 = mybir.dt.float32
AF = mybir.ActivationFunctionType
ALU = mybir.AluOpType
bf16 = ml_dtypes.bfloat16

P = 128
NCORES = 8
N_FULL = 4096
NIN, E1, E3, NZ, K = 2000, 500, 2000, 10, 10
NINP, EP, E3P, NZP = 2048, 512, 2048, 16       # padded dims
NIN_CH, E_CH, E3_CH = NINP // P, EP // P, E3P // P   # 16, 4, 16
CATP = EP + EP + E3P + P                        # 3200: [z1|z2|z3|z] padded
CAT_CH = CATP // P                              # 25

_BUILD_CACHE = {}


def build_graph(N=N_FULL):
    R = N // NCORES          # rows per core
    NB = R // P              # node blocks per core
    CC = N // P              # contraction (column) chunks for spmm

    nc = bacc.Bacc("TRN2", target_bir_lowering=False, debug=False,
                   num_devices=NCORES)

    def din(name, shape, dt=BF):
        return nc.declare_dram_parameter(name, list(shape), dt, isOutput=False)

    def dout(name, shape):
        return nc.declare_dram_parameter(name, list(shape), F32, isOutput=True)

    adjT_p = din("adjT", [N, R])
    xT_p = din("xT", [NINP, R])
    wp = {}
    for nm, sh in [
        ("enc1_w", (NINP, EP)), ("enc2_w", (EP, EP)), ("enc3_w", (EP, E3P)),
        ("zl_w", (E3P, NZP)), ("dec1_w", (P, NINP)), ("dec2_w", (NINP, EP)),
        ("dec3_w", (EP, EP)), ("xbar_w", (EP, NINP)),
        ("g1_w", (NINP, EP)), ("g2_w", (EP, EP)), ("g3_w", (EP, E3P)),
        ("g4_w", (E3P, NZP)), ("g6_w", (P, NINP)), ("g7_w", (NINP, EP)),
        ("g8_w", (EP, EP)), ("g9_w", (EP, NINP)),
        ("agnn_w", (CATP, NZP)), ("m1_w", (2 * EP, 2)), ("m2_w", (2 * EP, 2)),
        ("m3_w", (2 * E3P, 2)), ("ml_w", (CATP, 4)), ("wq", (P, NZP)),
    ]:
        wp[nm] = din(nm, sh)
    bp = {}
    for nm, sh in [
        ("enc1_b", (P, E_CH)), ("enc2_b", (P, E_CH)), ("enc3_b", (P, E3_CH)),
        ("zl_b", (NZP, 1)), ("dec1_b", (P, NIN_CH)), ("dec2_b", (P, E_CH)),
        ("dec3_b", (P, E_CH)), ("xbar_b", (P, NIN_CH)),
        ("m1_b", (2, 1)), ("m2_b", (2, 1)), ("m3_b", (2, 1)), ("ml_b", (4, 1)),
    ]:
        bp[nm] = din(nm, sh, F32)

    o_xbarT = dout("o_xbarT", [NINP, R])
    o_zhatT = dout("o_zhatT", [NINP, R])
    o_adjhat = dout("o_adjhat", [R, N])
    o_small = {nm: dout("o_" + nm, [R, NZP])
               for nm in ("q", "q1", "z", "r", "zl", "pred")}

    rg = [list(range(NCORES))]

    from contextlib import ExitStack
    with tile.TileContext(nc) as tc, ExitStack() as stack:
        pers = stack.enter_context(tc.tile_pool(name="pers", bufs=1))
        wst = stack.enter_context(tc.tile_pool(name="wst", bufs=3))
        tst = stack.enter_context(tc.tile_pool(name="tst", bufs=4))
        tmp = stack.enter_context(tc.tile_pool(name="tmp", bufs=2))
        sbst = stack.enter_context(tc.tile_pool(name="sbst", bufs=3))
        tlocp = stack.enter_context(tc.tile_pool(name="tlocp", bufs=1))
        ps = stack.enter_context(tc.tile_pool(name="ps", bufs=8, space="PSUM"))
        dram = stack.enter_context(tc.tile_pool(name="dram", bufs=1, space="DRAM"))

        _nmc = [0]

        def uname(pfx):
            _nmc[0] += 1
            return f"{pfx}{_nmc[0]}"

        # ---------------- constants / params in SBUF ----------------
        ident = pers.tile([P, P], BF, tag="ident")
        make_identity(nc, ident[:])
        ones_f = pers.tile([P, 1], F32, tag="ones_f")
        nc.gpsimd.memset(ones_f[:], 1.0)
        ones_row = pers.tile([1, 512], BF, tag="ones_row")
        nc.gpsimd.memset(ones_row[:], 1.0)

        adjT = pers.tile([P, CC, R], BF, tag="adjT")
        nc.sync.dma_start(adjT[:], adjT_p.ap().rearrange("(po pi) f -> pi po f", pi=P))

        bias = {}
        for nm in bp:
            t = pers.tile(list(bp[nm].shape), F32, tag="b_" + nm, name="b_" + nm)
            nc.sync.dma_start(t[:], bp[nm].ap())
            bias[nm] = t

        def resident_w(nm, nch, nl):
            t = pers.tile([P, nch, nl], BF, tag="w_" + nm, name="w_" + nm)
            nc.sync.dma_start(t[:], wp[nm].ap().rearrange("(po pi) l -> pi po l", pi=P))
            return t

        m1w = resident_w("m1_w", 8, 2)
        m2w = resident_w("m2_w", 8, 2)
        m3w = resident_w("m3_w", 32, 2)
        mlw = resident_w("ml_w", CAT_CH, 4)
        agnnw = resident_w("agnn_w", CAT_CH, NZP)
        zlw = resident_w("zl_w", E3_CH, NZP)
        g4w = resident_w("g4_w", E3_CH, NZP)
        wqw = resident_w("wq", 1, NZP)
        dec1w = resident_w("dec1_w", 1, NINP)
        g6w = resident_w("g6_w", 1, NINP)

        # ---------------- helpers ----------------
        def wtile(param, k, f0, fsz):
            t = wst.tile([P, fsz], BF, tag="w", name=uname("wt"))
            nc.scalar.dma_start(t[:], param.ap()[k * P:(k + 1) * P, f0:f0 + fsz])
            return t

        def psum(shape, dt=F32):
            return ps.tile(list(shape), dt, tag="ps", name=uname("ps"))

        def sb_chunk(src_dram, c):
            """stream one feature-major [128, R] bf16 chunk back from DRAM"""
            t = sbst.tile([P, R], BF, tag="sb", name=uname("sb"))
            nc.scalar.dma_start(t[:], src_dram[c * P:(c + 1) * P, :])
            return t[:]

        def fmajor_linear(dst, src_fn, wparam, nk, nfo_total, bias_t=None,
                          act=AF.Relu, dst_dram=None):
            """feature-major linear: out = act(W.T @ src + b).
            dst: [128, nfo, R] sbuf or None; dst_dram: [nfo*128, R] dram spill.
            src_fn(k) -> [128, R] AP."""
            nfo = nfo_total // P
            for g0 in range(0, nfo, 4):
                gn = min(4, nfo - g0)
                pss = [psum([P, R]) for _ in range(gn)]
                for k_ in range(nk):
                    wt = wtile(wparam, k_, g0 * P, gn * P)
                    src = src_fn(k_)
                    for j in range(gn):
                        nc.tensor.matmul(pss[j][:], wt[:, j * P:(j + 1) * P],
                                         src, start=(k_ == 0), stop=(k_ == nk - 1))
                for j in range(gn):
                    fo = g0 + j
                    b_ap = bias_t[:, fo:fo + 1] if bias_t is not None else 0.0
                    if dst is not None:
                        nc.scalar.activation(dst[:, fo, :], pss[j][:], act, bias=b_ap)
                    if dst_dram is not None:
                        ot = sbst.tile([P, R], BF, tag="sb", name=uname("fo"))
                        nc.scalar.activation(ot[:], pss[j][:], act, bias=b_ap)
                        nc.gpsimd.dma_start(dst_dram[fo * P:(fo + 1) * P, :], ot[:])

        def nmajor_linear(dst, src_fn, wparam, nk, fo_total, wtiles=None,
                          after_fog=None):
            """node-major inner product: dst [128, NB, fo_total] = src.T @ W.
            src_fn(k) -> feature-major [128, R] chunk. after_fog: {fog: cb}."""
            for fog in range((fo_total + 511) // 512):
                fsz = min(512, fo_total - fog * 512)
                pss = [psum([P, fsz]) for _ in range(NB)]
                for k_ in range(nk):
                    if wtiles is not None:
                        wt = wtiles[:, k_, fog * 512:fog * 512 + fsz]
                    else:
                        wt = wtile(wparam, k_, fog * 512, fsz)[:]
                    src = src_fn(k_)
                    for nb in range(NB):
                        nc.tensor.matmul(pss[nb][:], src[:, nb * P:(nb + 1) * P],
                                         wt, start=(k_ == 0), stop=(k_ == nk - 1))
                for nb in range(NB):
                    nc.vector.tensor_copy(dst[:, nb, fog * 512:fog * 512 + fsz],
                                          pss[nb][:])
                if after_fog and fog in after_fog:
                    after_fog[fog]()

        def allgather(loc, w, name, col0=0, wsub=None):
            """loc [128, NB, w] bf16 node-major -> gathered [N, wsub] dram,
            gathering only columns [col0, col0+wsub)."""
            wsub = wsub or w
            bounce = dram.tile([R, wsub], BF, tag="b_" + name, name="b_" + name)
            nc.gpsimd.dma_start(bounce[:].rearrange("(po pi) f -> pi po f", pi=P),
                              loc[:, :, col0:col0 + wsub])
            full = dram.tile([N, wsub], BF, tag="g_" + name, name="g_" + name,
                             addr_space="Shared")
            nc.gpsimd.collective_compute(
                "AllGather", ALU.bypass, replica_groups=rg,
                ins=[bounce[:].opt()], outs=[full[:].opt()])
            return full

        def spmm_pass(pss, tfull, pw):
            """one spmm accumulation pass; tfull [N, pw*128], pss: pw psums."""
            for c2 in range(CC // 2):
                tt = tst.tile([P, 2, pw * P], BF, tag="t", name=uname("t"))
                nc.sync.dma_start(
                    tt[:], tfull[c2 * 2 * P:(c2 + 1) * 2 * P, :]
                    .rearrange("(po pi) f -> pi po f", pi=P))
                for i2 in range(2):
                    col = c2 * 2 + i2
                    for j in range(pw):
                        nc.tensor.matmul(pss[j][:], tt[:, i2, j * P:(j + 1) * P],
                                         adjT[:, col, :], start=(col == 0),
                                         stop=(col == CC - 1))

        def spmm_f(dstT, fulls, w_total, act, dst_dram=None, fp32_dram=None,
                   epilogue=None):
            """feature-major spmm over pass-aligned gathered tensors.
            fulls: list of (dram_tensor, pass_width_chunks)."""
            fo = 0
            for tfull, pw in fulls:
                pss = [psum([P, R]) for _ in range(pw)]
                spmm_pass(pss, tfull, pw)
                for j in range(pw):
                    if fp32_dram is not None:
                        xo = tmp.tile([P, R], F32, tag="fo32", name=uname("fo32"),
                                      bufs=2)
                        nc.scalar.activation(xo[:], pss[j][:], act)
                        nc.sync.dma_start(fp32_dram[fo * P:(fo + 1) * P, :], xo[:])
                        if dstT is not None:
                            nc.vector.tensor_copy(dstT[:, fo, :], xo[:])
                    elif dstT is not None:
                        nc.scalar.activation(dstT[:, fo, :], pss[j][:], act)
                    elif dst_dram is not None:
                        ot = sbst.tile([P, R], BF, tag="sb", name=uname("so"))
                        nc.scalar.activation(ot[:], pss[j][:], act)
                        nc.gpsimd.dma_start(dst_dram[fo * P:(fo + 1) * P, :], ot[:])
                    fo += 1
                if epilogue is not None:
                    epilogue(fo)

        def spmm_narrow(tfull):
            """narrow spmm: load whole [N, NZP] in one DMA, one psum out."""
            tt = tst.tile([P, CC, NZP], BF, tag="tn", name=uname("tn"))
            nc.sync.dma_start(tt[:],
                              tfull[:].rearrange("(po pi) f -> pi po f", pi=P))
            pp = psum([NZP, R])
            for col in range(CC):
                nc.tensor.matmul(pp[:], tt[:, col, :], adjT[:, col, :],
                                 start=(col == 0), stop=(col == CC - 1))
            return pp

        def gate(src_list, wres, nktot, bias_t, nl):
            """softmax(lrelu(cat @ w + b)) row-L2-normalized.
            src_list: [(chunk_fn, nch)]; returns nl fp32 [128,R] bcast tiles."""
            pp = psum([nl, R])
            k_ = 0
            for fn, nch in src_list:
                for c in range(nch):
                    nc.tensor.matmul(pp[:], wres[:, k_, :nl], fn(c),
                                     start=(k_ == 0), stop=(k_ == nktot - 1))
                    k_ += 1
            assert k_ == nktot
            e_full = tmp.tile([P, R], F32, tag="gt_e", name=uname("gt_e"), bufs=1)
            nc.vector.memset(e_full[:], 0.0)
            lin = tmp.tile([nl, R], F32, tag="gt_lin", name=uname("gt_lin"), bufs=1)
            nc.scalar.activation(lin[:], pp[:], AF.Identity, bias=bias_t[:, 0:1])
            sc = tmp.tile([nl, R], F32, tag="gt_sc", name=uname("gt_sc"), bufs=1)
            nc.vector.tensor_scalar_mul(sc[:], lin[:], 0.01)
            lr = tmp.tile([nl, R], F32, tag="gt_lr", name=uname("gt_lr"), bufs=1)
            nc.vector.tensor_max(lr[:], lin[:], sc[:])
            nc.scalar.activation(e_full[0:nl, :], lr[:], AF.Exp)
            s1 = psum([1, R])
            nc.tensor.matmul(s1[:], ones_f[:, 0:1], e_full[:], start=True, stop=True)
            rinv = tmp.tile([1, R], F32, tag="gt_rinv", name=uname("gt_rinv"), bufs=1)
            nc.vector.reciprocal(rinv[:], s1[:])
            rb = tmp.tile([P, R], F32, tag="gt_rb", name=uname("gt_rb"), bufs=1)
            nc.gpsimd.partition_broadcast(rb[:], rinv[:])
            p_t = tmp.tile([P, R], F32, tag="gt_p", name=uname("gt_p"), bufs=1)
            nc.vector.memset(p_t[:], 0.0)
            nc.vector.tensor_mul(p_t[0:nl, :], e_full[0:nl, :], rb[0:nl, :])
            sq = tmp.tile([P, R], F32, tag="gt_sq", name=uname("gt_sq"), bufs=1)
            nc.vector.memset(sq[:], 0.0)
            nc.vector.tensor_mul(sq[0:nl, :], p_t[0:nl, :], p_t[0:nl, :])
            n2 = psum([1, R])
            nc.tensor.matmul(n2[:], ones_f[:, 0:1], sq[:], start=True, stop=True)
            sqr = tmp.tile([1, R], F32, tag="gt_sqr", name=uname("gt_sqr"), bufs=1)
            nc.scalar.activation(sqr[:], n2[:], AF.Sqrt)
            rn = tmp.tile([1, R], F32, tag="gt_rn", name=uname("gt_rn"), bufs=1)
            nc.vector.reciprocal(rn[:], sqr[:])
            rnb = tmp.tile([P, R], F32, tag="gt_rnb", name=uname("gt_rnb"), bufs=1)
            nc.gpsimd.partition_broadcast(rnb[:], rn[:])
            nc.vector.tensor_mul(p_t[0:nl, :], p_t[0:nl, :], rnb[0:nl, :])
            outs = []
            for i in range(nl):
                row = tmp.tile([1, R], F32, tag=f"gt_row{i}", name=uname("gt_row"),
                               bufs=1)
                if i == 0:
                    nc.vector.tensor_copy(row[:], p_t[0:1, :])
                else:
                    nc.sync.dma_start(row[:], p_t[i:i + 1, :])
                pb = tmp.tile([P, R], F32, tag=f"gt_pb{i}", name=uname("gt_pb"),
                              bufs=1)
                nc.gpsimd.partition_broadcast(pb[:], row[:])
                outs.append(pb)
            return outs

        def mix_chunk(a_ap, b_ap, pa, pb_, out_ap=None):
            """one [128,R] bf16 chunk of pa*a + pb*b"""
            t0 = tmp.tile([P, R], F32, tag="mx0", name=uname("mx0"))
            t1 = tmp.tile([P, R], F32, tag="mx1", name=uname("mx1"))
            if out_ap is None:
                out_ap = tmp.tile([P, R], BF, tag="mxo", name=uname("mxo"))[:]
            nc.vector.tensor_mul(t0[:], a_ap, pa[:])
            nc.vector.tensor_mul(t1[:], b_ap, pb_[:])
            nc.vector.tensor_add(out_ap, t0[:], t1[:])
            return out_ap

        def narrow_nm(srcT, bf_dst=None, f32_dram=None, want_f32=False):
            f32s = []
            for nb in range(NB):
                pt = psum([P, NZP], BF)
                nc.tensor.transpose(pt[:], srcT[0:NZP, nb * P:(nb + 1) * P],
                                    ident[0:NZP, 0:NZP])
                if bf_dst is not None:
                    nc.vector.tensor_copy(bf_dst[:, nb, :], pt[:])
                ft = None
                if want_f32 or f32_dram is not None:
                    ft = tmp.tile([P, NZP], F32, tag="nmf", name=uname("nmf"),
                                  bufs=NB + 1)
                    nc.vector.tensor_copy(ft[:], pt[:])
                    if f32_dram is not None:
                        nc.sync.dma_start(f32_dram[nb * P:(nb + 1) * P, :], ft[:])
                f32s.append(ft)
            return f32s

        def student_t(lhsT_aug, nm_f32s, out_dram):
            for nb in range(NB):
                pq = psum([P, NZP])
                nc.tensor.matmul(pq[:], lhsT_aug[:, nb * P:(nb + 1) * P],
                                 wqw[:, 0, :], start=True, stop=True)
                sqv = tmp.tile([P, NZ], F32, tag="q_sq", name=uname("q_sq"))
                nc.vector.tensor_mul(sqv[:], nm_f32s[nb][:, 0:NZ],
                                     nm_f32s[nb][:, 0:NZ])
                zn = tmp.tile([P, 1], F32, tag="q_zn", name=uname("q_zn"))
                nc.vector.tensor_reduce(zn[:], sqv[:], mybir.AxisListType.X, ALU.add)
                d2 = tmp.tile([P, NZ], F32, tag="q_d2", name=uname("q_d2"))
                nc.vector.tensor_scalar(d2[:], pq[:, 0:NZ], zn[:], None, ALU.add)
                qn = tmp.tile([P, NZ], F32, tag="q_qn", name=uname("q_qn"))
                nc.vector.reciprocal(qn[:], d2[:])
                sm = tmp.tile([P, 1], F32, tag="q_sm", name=uname("q_sm"))
                nc.vector.tensor_reduce(sm[:], qn[:], mybir.AxisListType.X, ALU.add)
                rs = tmp.tile([P, 1], F32, tag="q_rs", name=uname("q_rs"))
                nc.vector.reciprocal(rs[:], sm[:])
                ot = tmp.tile([P, NZP], F32, tag="q_ot", name=uname("q_ot"))
                nc.vector.memset(ot[:], 0.0)
                nc.vector.tensor_scalar(ot[:, 0:NZ], qn[:], rs[:], None, ALU.mult)
                nc.gpsimd.dma_start(out_dram[nb * P:(nb + 1) * P, :], ot[:])

        # ====================================================================
        # Schedule: the GNN chain is serial through 11 AllGathers; all
        # independent work (AE encoder/decoder, xbar, gates, student-t) is
        # placed to fill specific gather-latency windows.
        # ====================================================================

        # ---- phase A: t1 = x @ g1 first, gather ASAP ----
        t1_loc = tlocp.tile([P, NB, EP], BF, tag="tloc", name=uname("tloc"), bufs=2)
        ps_t1 = [psum([P, EP]) for _ in range(NB)]
        for k2 in range(NIN_CH // 2):
            xt2 = tst.tile([P, 2, R], BF, tag="t", name=uname("xt"))
            nc.sync.dma_start(xt2[:], xT_p.ap()[k2 * 2 * P:(k2 + 1) * 2 * P, :]
                              .rearrange("(po pi) f -> pi po f", pi=P))
            for i2 in range(2):
                k_ = k2 * 2 + i2
                g1t = wtile(wp["g1_w"], k_, 0, EP)
                for nb in range(NB):
                    nc.tensor.matmul(ps_t1[nb][:],
                                     xt2[:, i2, nb * P:(nb + 1) * P],
                                     g1t[:], start=(k_ == 0),
                                     stop=(k_ == NIN_CH - 1))
        for nb in range(NB):
            nc.vector.tensor_copy(t1_loc[:, nb, :], ps_t1[nb][:])
        t1_full = allgather(t1_loc, EP, "t1")

        # [AG-t1 window] enc1 + enc2
        r_e1T = pers.tile([P, E_CH, R], BF, tag="r_e1T")
        ps_e1 = [psum([P, R]) for _ in range(E_CH)]
        for k2 in range(NIN_CH // 2):
            xt2 = tst.tile([P, 2, R], BF, tag="t", name=uname("xt"))
            nc.sync.dma_start(xt2[:], xT_p.ap()[k2 * 2 * P:(k2 + 1) * 2 * P, :]
                              .rearrange("(po pi) f -> pi po f", pi=P))
            for i2 in range(2):
                k_ = k2 * 2 + i2
                e1t = wtile(wp["enc1_w"], k_, 0, EP)
                for fo in range(E_CH):
                    nc.tensor.matmul(ps_e1[fo][:], e1t[:, fo * P:(fo + 1) * P],
                                     xt2[:, i2, :], start=(k_ == 0),
                                     stop=(k_ == NIN_CH - 1))
        for fo in range(E_CH):
            nc.scalar.activation(r_e1T[:, fo, :], ps_e1[fo][:], AF.Relu,
                                 bias=bias["enc1_b"][:, fo:fo + 1])
        r_e2T = pers.tile([P, E_CH, R], BF, tag="r_e2T")
        fmajor_linear(r_e2T, lambda k: r_e1T[:, k, :], wp["enc2_w"], E_CH, EP,
                      bias["enc2_b"])

        # ---- z1 ----
        z1T = pers.tile([P, E_CH, R], BF, tag="z1T")
        spmm_f(z1T, [(t1_full, 4)], EP, AF.Relu)
        p1 = gate([(lambda c: r_e1T[:, c, :], E_CH), (lambda c: z1T[:, c, :], E_CH)],
                  m1w, 8, bias["m1_b"], 2)
        t2_loc = tlocp.tile([P, NB, EP], BF, tag="tloc", name=uname("tloc"), bufs=2)
        nmajor_linear(t2_loc, lambda k: mix_chunk(z1T[:, k, :], r_e1T[:, k, :],
                                                  p1[0], p1[1]),
                      wp["g2_w"], E_CH, EP)
        t2_full = allgather(t2_loc, EP, "t2")

        # [AG-t2 window] r_e3 (fused with r = r_e3 @ zl_w + zl_b), then q1/r
        pr = psum([NZP, R])
        for g0 in range(0, E3_CH, 4):
            pss = [psum([P, R]) for _ in range(4)]
            for k_ in range(E_CH):
                wt = wtile(wp["enc3_w"], k_, g0 * P, 4 * P)
                for j in range(4):
                    nc.tensor.matmul(pss[j][:], wt[:, j * P:(j + 1) * P],
                                     r_e2T[:, k_, :], start=(k_ == 0),
                                     stop=(k_ == E_CH - 1))
            for j in range(4):
                fo = g0 + j
                d3t = sbst.tile([P, R], BF, tag="sb", name=uname("re3"))
                nc.scalar.activation(d3t[:], pss[j][:], AF.Relu,
                                     bias=bias["enc3_b"][:, fo:fo + 1])
                nc.tensor.matmul(pr[:], zlw[:, fo, :], d3t[:],
                                 start=(fo == 0), stop=(fo == E3_CH - 1))
        rT = pers.tile([P, R], BF, tag="rT")
        nc.gpsimd.memset(rT[:], 0.0)
        nc.scalar.activation(rT[0:NZP, :], pr[:], AF.Identity,
                             bias=bias["zl_b"][:, 0:1])
        nc.sync.dma_start(rT[NZ:NZ + 1, :], ones_row[0:1, 0:R])  # student-t aug
        r_nm_bf = pers.tile([P, NB, NZP], BF, tag="r_nm")
        r_f32s = narrow_nm(rT, bf_dst=r_nm_bf, f32_dram=o_small["r"].ap(),
                           want_f32=True)
        student_t(rT, r_f32s, o_small["q1"].ap())

        # ---- z2 ----
        z2T = pers.tile([P, E_CH, R], BF, tag="z2T")
        spmm_f(z2T, [(t2_full, 4)], EP, AF.Relu)
        # t3 = z2 @ g3 with half gathers
        t3_loc = tlocp.tile([P, NB, E3P], BF, tag="tlocbig", name=uname("tloc"))
        t3_h = []
        nmajor_linear(t3_loc, lambda k: z2T[:, k, :], wp["g3_w"], E_CH, E3P,
                      after_fog={1: lambda: t3_h.append(
                                     allgather(t3_loc, E3P, "t3a", 0, 1024)),
                                 3: lambda: t3_h.append(
                                     allgather(t3_loc, E3P, "t3b", 1024, 1024))})

        # [AG-t3 window] m2 gate, mix2, h3 -> DRAM
        p2 = gate([(lambda c: r_e2T[:, c, :], E_CH), (lambda c: z2T[:, c, :], E_CH)],
                  m2w, 8, bias["m2_b"], 2)
        h3_d = dram.tile([E3P, R], BF, tag="h3_d", name="h3_d")
        mix2T = pers.tile([P, E_CH, R], BF, tag="mix2T")
        for c in range(E_CH):
            mix_chunk(z2T[:, c, :], r_e2T[:, c, :], p2[0], p2[1],
                      out_ap=mix2T[:, c, :])
        fmajor_linear(None, lambda k: mix2T[:, k, :], wp["enc3_w"], E_CH, E3P,
                      bias["enc3_b"], dst_dram=h3_d[:])

        # ---- z3 (to DRAM) ----
        z3_d = dram.tile([E3P, R], BF, tag="z3_d", name="z3_d")
        spmm_f(None, [(t3_h[0], 8), (t3_h[1], 8)], E3P, AF.Relu,
               dst_dram=z3_d[:])

        # ---- m3 gate, t4 = mix3 @ g4 ----
        p3 = gate([(lambda c: sb_chunk(h3_d[:], c), E3_CH),
                   (lambda c: sb_chunk(z3_d[:], c), E3_CH)],
                  m3w, 32, bias["m3_b"], 2)
        t4_loc = pers.tile([P, NB, NZP], BF, tag="t4_loc")
        nmajor_linear(t4_loc, lambda k: mix_chunk(sb_chunk(z3_d[:], k),
                                                  sb_chunk(h3_d[:], k),
                                                  p3[0], p3[1]),
                      wp["g4_w"], E3_CH, NZP, wtiles=g4w)
        t4_full = allgather(t4_loc, NZP, "t4")

        # [AG-t4 window] dec1+dec2 fused
        r_d2T = pers.tile([P, E_CH, R], BF, tag="r_d2T")
        ps_d2 = [psum([P, R]) for _ in range(E_CH)]
        for k_ in range(NIN_CH):
            pd1 = psum([P, R])
            nc.tensor.matmul(pd1[:], dec1w[:, 0, k_ * P:(k_ + 1) * P], rT[:],
                             start=True, stop=True)
            d1t = tmp.tile([P, R], BF, tag="d1t", name=uname("d1t"))
            nc.scalar.activation(d1t[:], pd1[:], AF.Relu,
                                 bias=bias["dec1_b"][:, k_:k_ + 1])
            w2t = wtile(wp["dec2_w"], k_, 0, EP)
            for fo in range(E_CH):
                nc.tensor.matmul(ps_d2[fo][:], w2t[:, fo * P:(fo + 1) * P], d1t[:],
                                 start=(k_ == 0), stop=(k_ == NIN_CH - 1))
        for fo in range(E_CH):
            nc.scalar.activation(r_d2T[:, fo, :], ps_d2[fo][:], AF.Relu,
                                 bias=bias["dec2_b"][:, fo:fo + 1])

        # ---- z = relu(adj @ t4), node-major z, z+r ----
        pz = spmm_narrow(t4_full)
        zT = pers.tile([P, R], BF, tag="zT")
        nc.gpsimd.memset(zT[:], 0.0)
        nc.scalar.activation(zT[0:NZP, :], pz[:], AF.Relu)
        zr_loc = pers.tile([P, NB, NZP], BF, tag="zr_loc")
        for nb in range(NB):
            pt = psum([P, NZP], BF)
            nc.tensor.transpose(pt[:], zT[0:NZP, nb * P:(nb + 1) * P],
                                ident[0:NZP, 0:NZP])
            nc.vector.tensor_add(zr_loc[:, nb, :], pt[:], r_nm_bf[:, nb, :])
            zf = tmp.tile([P, NZP], F32, tag="nmf", name=uname("zf"), bufs=NB + 1)
            nc.vector.tensor_copy(zf[:], pt[:])
            nc.gpsimd.dma_start(o_small["z"].ap()[nb * P:(nb + 1) * P, :], zf[:])
        zr_full = allgather(zr_loc, NZP, "zr")

        # [zr window] dec3
        r_d3T = pers.tile([P, E_CH, R], BF, tag="r_d3T")
        fmajor_linear(r_d3T, lambda k: r_d2T[:, k, :], wp["dec3_w"], E_CH, EP,
                      bias["dec3_b"])

        # ---- t6 = z @ g6, halves gathered ----
        t6_loc = tlocp.tile([P, NB, NINP], BF, tag="tlocbig", name=uname("tloc"))
        t6_h = []
        for fog in range(NIN_CH // 4):
            pss = [psum([P, 512]) for _ in range(NB)]
            for nb in range(NB):
                nc.tensor.matmul(pss[nb][:], zT[:, nb * P:(nb + 1) * P],
                                 g6w[:, 0, fog * 512:(fog + 1) * 512],
                                 start=True, stop=True)
            for nb in range(NB):
                nc.vector.tensor_copy(t6_loc[:, nb, fog * 512:(fog + 1) * 512],
                                      pss[nb][:])
            if fog == 1:
                t6_h.append(allgather(t6_loc, NINP, "t6a", 0, 1024))
            elif fog == 3:
                t6_h.append(allgather(t6_loc, NINP, "t6b", 1024, 1024))

        # [AG-t6 window] xbar groups 0,1 + z_l + q
        def xbar_group(g0):
            pss = [psum([P, R]) for _ in range(4)]
            for k_ in range(E_CH):
                wt = wtile(wp["xbar_w"], k_, g0 * P, 4 * P)
                for j in range(4):
                    nc.tensor.matmul(pss[j][:], wt[:, j * P:(j + 1) * P],
                                     r_d3T[:, k_, :], start=(k_ == 0),
                                     stop=(k_ == E_CH - 1))
            for j in range(4):
                fo = g0 + j
                xo = tmp.tile([P, R], F32, tag="fo32", name=uname("xo"), bufs=2)
                nc.scalar.activation(xo[:], pss[j][:], AF.Identity,
                                     bias=bias["xbar_b"][:, fo:fo + 1])
                nc.gpsimd.dma_start(o_xbarT.ap()[fo * P:(fo + 1) * P, :], xo[:])

        xbar_group(0)
        pzl = spmm_narrow(zr_full)
        z_lT = pers.tile([P, R], BF, tag="z_lT")
        nc.gpsimd.memset(z_lT[:], 0.0)
        nc.vector.tensor_copy(z_lT[0:NZP, :], pzl[:])
        nc.sync.dma_start(z_lT[NZ:NZ + 1, :], ones_row[0:1, 0:R])
        zl_f32s = narrow_nm(z_lT, f32_dram=o_small["zl"].ap(), want_f32=True)
        student_t(z_lT, zl_f32s, o_small["q"].ap())
        xbar_group(4)

        # ---- dz1 (to DRAM) ----
        dz1_d = dram.tile([NINP, R], BF, tag="dz1_d", name="dz1_d")
        spmm_f(None, [(t6_h[0], 8), (t6_h[1], 8)], NINP, AF.Relu,
               dst_dram=dz1_d[:])
        t7_loc = tlocp.tile([P, NB, EP], BF, tag="tloc", name=uname("tloc"), bufs=2)
        nmajor_linear(t7_loc, lambda k: sb_chunk(dz1_d[:], k), wp["g7_w"],
                      NIN_CH, EP)
        t7_full = allgather(t7_loc, EP, "t7")

        # [AG-t7 window] ml gate + t_a + its gather, xbar group 2
        zT3 = zT.rearrange("p (c f) -> p c f", c=1)
        pml = gate([(lambda c: z1T[:, c, :], E_CH), (lambda c: z2T[:, c, :], E_CH),
                    (lambda c: sb_chunk(z3_d[:], c), E3_CH),
                    (lambda c: zT3[:, 0, :], 1)],
                   mlw, CAT_CH, bias["ml_b"], 4)
        ta_loc = pers.tile([P, NB, NZP], BF, tag="ta_loc")
        ps_ta = [psum([P, NZP]) for _ in range(NB)]
        blocks = [(lambda c: z1T[:, c, :], E_CH, pml[0]),
                  (lambda c: z2T[:, c, :], E_CH, pml[1]),
                  (lambda c: sb_chunk(z3_d[:], c), E3_CH, pml[2]),
                  (lambda c: zT3[:, 0, :], 1, pml[3])]
        k_ = 0
        for fn, nch, pb_ in blocks:
            for c in range(nch):
                mz = tmp.tile([P, R], BF, tag="mz", name=uname("mz"))
                nc.vector.tensor_mul(mz[:], fn(c), pb_[:])
                for nb in range(NB):
                    nc.tensor.matmul(ps_ta[nb][:], mz[:, nb * P:(nb + 1) * P],
                                     agnnw[:, k_, :], start=(k_ == 0),
                                     stop=(k_ == CAT_CH - 1))
                k_ += 1
        for nb in range(NB):
            nc.vector.tensor_copy(ta_loc[:, nb, :], ps_ta[nb][:])
        ta_full = allgather(ta_loc, NZP, "ta")
        xbar_group(8)

        # ---- dz2 ----
        dz2T = pers.tile([P, E_CH, R], BF, tag="dz2T")
        spmm_f(dz2T, [(t7_full, 4)], EP, AF.Relu)
        t8_loc = tlocp.tile([P, NB, EP], BF, tag="tloc", name=uname("tloc"), bufs=2)
        nmajor_linear(t8_loc, lambda k: dz2T[:, k, :], wp["g8_w"], E_CH, EP)
        t8_full = allgather(t8_loc, EP, "t8")

        # [AG-t8 window] pred = softmax(adj @ t_a), xbar group 3
        tta = tst.tile([P, CC, NZP], BF, tag="tn", name=uname("tta"))
        nc.sync.dma_start(tta[:],
                          ta_full[:].rearrange("(po pi) f -> pi po f", pi=P))
        ps_pred = [psum([P, NZP]) for _ in range(NB)]
        for col in range(CC):
            for nb in range(NB):
                nc.tensor.matmul(ps_pred[nb][:], adjT[:, col, nb * P:(nb + 1) * P],
                                 tta[:, col, :], start=(col == 0),
                                 stop=(col == CC - 1))
        for nb in range(NB):
            ex = tmp.tile([P, NZ], F32, tag="pr_e", name=uname("pr_e"))
            s = tmp.tile([P, 1], F32, tag="pr_s", name=uname("pr_s"))
            nc.scalar.activation(ex[:], ps_pred[nb][:, 0:NZ], AF.Exp, accum_out=s[:])
            ri = tmp.tile([P, 1], F32, tag="pr_ri", name=uname("pr_ri"))
            nc.vector.reciprocal(ri[:], s[:])
            po = tmp.tile([P, NZP], F32, tag="pr_o", name=uname("pr_o"))
            nc.vector.memset(po[:], 0.0)
            nc.vector.tensor_scalar(po[:, 0:NZ], ex[:], ri[:], None, ALU.mult)
            nc.gpsimd.dma_start(o_small["pred"].ap()[nb * P:(nb + 1) * P, :], po[:])
        xbar_group(12)

        # ---- dz3 ----
        dz3T = pers.tile([P, E_CH, R], BF, tag="dz3T")
        spmm_f(dz3T, [(t8_full, 4)], EP, AF.Relu)
        t9_loc = tlocp.tile([P, NB, NINP], BF, tag="tlocbig", name=uname("tloc"))
        t9_h = []
        nmajor_linear(t9_loc, lambda k: dz3T[:, k, :], wp["g9_w"], E_CH, NINP,
                      after_fog={1: lambda: t9_h.append(
                                     allgather(t9_loc, NINP, "t9a", 0, 1024)),
                                 3: lambda: t9_h.append(
                                     allgather(t9_loc, NINP, "t9b", 1024, 1024))})

        # ---- z_hat: spmm halves, each followed by its zh gather half ----
        z_hatT = pers.tile([P, NIN_CH, R], BF, tag="z_hatT")
        zh_bounce = dram.tile([NINP, R], BF, tag="b_zh", name="b_zh")
        zh_h = []
        for p_i in range(2):
            pss = [psum([P, R]) for _ in range(8)]
            spmm_pass(pss, t9_h[p_i], 8)
            for j in range(8):
                fo = p_i * 8 + j
                xo = tmp.tile([P, R], F32, tag="fo32", name=uname("fo32"), bufs=2)
                nc.scalar.activation(xo[:], pss[j][:], AF.Relu)
                nc.gpsimd.dma_start(o_zhatT.ap()[fo * P:(fo + 1) * P, :], xo[:])
                nc.vector.tensor_copy(z_hatT[:, fo, :], xo[:])
                nc.gpsimd.dma_start(zh_bounce[fo * P:(fo + 1) * P, :],
                                  z_hatT[:, fo, :])
            full = dram.tile([NCORES * 1024, R], BF, tag=f"g_zh{p_i}",
                             name=f"g_zh{p_i}", addr_space="Shared")
            nc.gpsimd.collective_compute(
                "AllGather", ALU.bypass, replica_groups=rg,
                ins=[zh_bounce[p_i * 1024:(p_i + 1) * 1024, :].opt()],
                outs=[full[:].opt()])
            zh_h.append(full)

        # ---- adj_hat = sigmoid(z_hat @ z_hat^T) ----
        for cb in range(NCORES):
            pss = [psum([P, R]) for _ in range(NB)]
            for f2 in range(NIN_CH // 2):
                h = f2 // 4
                base = cb * 1024 + (f2 % 4) * 256
                rt2 = tst.tile([P, 2, R], BF, tag="t", name=uname("rt"))
                nc.sync.dma_start(rt2[:], zh_h[h][base:base + 256, :]
                                  .rearrange("(po pi) f -> pi po f", pi=P))
                for i2 in range(2):
                    f = f2 * 2 + i2
                    for nb in range(NB):
                        nc.tensor.matmul(pss[nb][:],
                                         z_hatT[:, f, nb * P:(nb + 1) * P],
                                         rt2[:, i2, :], start=(f == 0),
                                         stop=(f == NIN_CH - 1))
            for nb in range(NB):
                so = tmp.tile([P, R], F32, tag="fo32", name=uname("ah"), bufs=2)
                nc.scalar.activation(so[:], pss[nb][:], AF.Sigmoid)
                nc.gpsimd.dma_start(o_adjhat.ap()[nb * P:(nb + 1) * P,
                                                cb * R:(cb + 1) * R], so[:])

    nc.compile()
    return nc


# ----------------------------------------------------------------------------
# host-side input prep
# ----------------------------------------------------------------------------

def _pad2(a, s0, s1):
    z = np.zeros((s0, s1), np.float32)
    z[:a.shape[0], :a.shape[1]] = a
    return z


def prep_in_maps(inputs, N=N_FULL):
    R = N // NCORES
    f32 = {k: np.asarray(v, np.float32) for k, v in inputs.items()}

    shared = {}
    for nm, s0, s1 in [
        ("enc1_w", NINP, EP), ("enc2_w", EP, EP), ("enc3_w", EP, E3P),
        ("zl_w", E3P, NZP), ("dec1_w", P, NINP), ("dec2_w", NINP, EP),
        ("dec3_w", EP, EP), ("xbar_w", EP, NINP),
        ("g1_w", NINP, EP), ("g2_w", EP, EP), ("g3_w", EP, E3P),
        ("g4_w", E3P, NZP), ("g6_w", P, NINP), ("g7_w", NINP, EP),
        ("g8_w", EP, EP), ("g9_w", EP, NINP),
    ]:
        shared[nm] = _pad2(f32[nm], s0, s1).astype(bf16)

    def blocks2(w, bsz, bpad):
        nb_ = w.shape[0] // bsz
        out = np.zeros((nb_ * bpad, w.shape[1]), np.float32)
        for i in range(nb_):
            out[i * bpad:i * bpad + bsz] = w[i * bsz:(i + 1) * bsz]
        return out

    shared["m1_w"] = blocks2(f32["m1_w"], E1, EP).astype(bf16)
    shared["m2_w"] = blocks2(f32["m2_w"], E1, EP).astype(bf16)
    shared["m3_w"] = blocks2(f32["m3_w"], E3, E3P).astype(bf16)

    def blocks_cat(w, ncols=None):
        out = np.zeros((CATP, ncols or w.shape[1]), np.float32)
        out[0:E1, :w.shape[1]] = w[0:E1]
        out[EP:EP + E1, :w.shape[1]] = w[E1:2 * E1]
        out[2 * EP:2 * EP + E3, :w.shape[1]] = w[2 * E1:2 * E1 + E3]
        out[2 * EP + E3P:2 * EP + E3P + NZ, :w.shape[1]] = w[2 * E1 + E3:2 * E1 + E3 + NZ]
        return out

    shared["ml_w"] = blocks_cat(f32["ml_w"]).astype(bf16)
    shared["agnn_w"] = blocks_cat(f32["agnn_w"], NZP).astype(bf16)

    cl = f32["cluster"]                      # [K, NZ]
    wq = np.zeros((P, NZP), np.float32)
    wq[0:NZ, 0:K] = -2.0 * cl.T
    wq[NZ, 0:K] = (cl * cl).sum(axis=1) + 1.0
    shared["wq"] = wq.astype(bf16)

    for nm, total, nch in [
        ("enc1_b", EP, E_CH), ("enc2_b", EP, E_CH), ("enc3_b", E3P, E3_CH),
        ("dec1_b", NINP, NIN_CH), ("dec2_b", EP, E_CH), ("dec3_b", EP, E_CH),
        ("xbar_b", NINP, NIN_CH),
    ]:
        b = np.zeros(total, np.float32)
        b[:f32[nm].shape[0]] = f32[nm]
        shared[nm] = np.ascontiguousarray(b.reshape(nch, P).T)
    shared["zl_b"] = _pad2(f32["zl_b"][:, None], NZP, 1)
    for nm, nl in [("m1_b", 2), ("m2_b", 2), ("m3_b", 2), ("ml_b", 4)]:
        shared[nm] = np.ascontiguousarray(f32[nm].reshape(nl, 1))

    adjT = np.ascontiguousarray(f32["adj"].T).astype(bf16)      # [N, N]
    xT = np.zeros((NINP, N), np.float32)
    xT[0:NIN] = f32["x"].T
    xT = xT.astype(bf16)

    in_maps = []
    for c in range(NCORES):
        m = dict(shared)
        m["adjT"] = np.ascontiguousarray(adjT[:, c * R:(c + 1) * R])
        m["xT"] = np.ascontiguousarray(xT[:, c * R:(c + 1) * R])
        in_maps.append(m)
    return in_maps


def assemble_outputs(results, N=N_FULL):
    def cat_rows(key):
        return np.concatenate([r[key] for r in results], axis=0)

    x_bar = np.concatenate([r["o_xbarT"][0:NIN, :].T for r in results], axis=0)
    z_hat = np.concatenate([r["o_zhatT"][0:NIN, :].T for r in results], axis=0)
    adj_hat = cat_rows("o_adjhat")
    q = cat_rows("o_q")[:, 0:NZ]
    q1 = cat_rows("o_q1")[:, 0:NZ]
    z = cat_rows("o_z")[:, 0:NZ]
    r_ = cat_rows("o_r")[:, 0:NZ]
    z_l = cat_rows("o_zl")[:, 0:NZ]
    pred = cat_rows("o_pred")[:, 0:NZ]
    return (x_bar, z_hat, adj_hat, q, q1, z, r_, z_l, pred)


def _run(inputs, trace=False):
    if N_FULL not in _BUILD_CACHE:
        _BUILD_CACHE[N_FULL] = build_graph(N_FULL)
    nc = _BUILD_CACHE[N_FULL]
    in_maps = prep_in_maps(inputs, N_FULL)
    res = run_bass_kernel_spmd(nc, in_maps, list(range(NCORES)), trace=trace)
    outs = assemble_outputs(res.results, N_FULL)
    return outs, res


def kernel(**inputs):
    outs, _ = _run(inputs, trace=False)
    return outs


# revision 34
# speedup vs baseline: 1.5353x; 1.3549x over previous
"""Trainium2 Bass kernel for nn_Adiin_24197845746021 (gnn_message_passing).

Row-shard the N=4096 nodes across 8 NeuronCores (512 rows each). Each core
holds adj[rows,:].T (bf16) resident in SBUF. adj @ X is computed as
lhsT = X_full tile (AllGathered, node-major), rhs = adjT shard ->
feature-major output. Feature matmuls consume feature-major activations
directly, so no transposes are needed except for the tiny [*,10] tensors.
All matmuls bf16 (fp32 PSUM); gating / softmax / student-t math fp32.
Contractions are zero-padded to multiples of 128 host-side. Wide (2000-dim)
mid-lived activations (z3, h3, dz1) spill to DRAM and are re-streamed.
"""

import numpy as np
import ml_dtypes

import concourse.bass as bass
import concourse.mybir as mybir
import concourse.tile as tile
from concourse import bacc
from concourse.bass_utils import run_bass_kernel_spmd
from concourse.masks import make_identity

BF = mybir.dt.bfloat16
F32 = mybir.dt.float32
AF = mybir.ActivationFunctionType
ALU = mybir.AluOpType
bf16 = ml_dtypes.bfloat16

P = 128
NCORES = 8
N_FULL = 4096
NIN, E1, E3, NZ, K = 2000, 500, 2000, 10, 10
NINP, EP, E3P, NZP = 2048, 512, 2048, 16       # padded dims
NIN_CH, E_CH, E3_CH = NINP // P, EP // P, E3P // P   # 16, 4, 16
CATP = EP + EP + E3P + P                        # 3200: [z1|z2|z3|z] padded
CAT_CH = CATP // P                              # 25

_BUILD_CACHE = {}


def build_graph(N=N_FULL):
    R = N // NCORES          # rows per core
    NB = R // P              # node blocks per core
    CC = N // P              # contraction (column) chunks for spmm

    nc = bacc.Bacc("TRN2", target_bir_lowering=False, debug=False,
                   num_devices=NCORES)

    def din(name, shape, dt=BF):
        return nc.declare_dram_parameter(name, list(shape), dt, isOutput=False)

    def dout(name, shape):
        return nc.declare_dram_parameter(name, list(shape), F32, isOutput=True)

    adjT_p = din("adjT", [N, R])
    xT_p = din("xT", [NINP, R])
    wp = {}
    for nm, sh in [
        ("enc1_w", (NINP, EP)), ("enc2_w", (EP, EP)), ("enc3_w", (EP, E3P)),
        ("zl_w", (E3P, NZP)), ("dec1_w", (P, NINP)), ("dec2_w", (NINP, EP)),
        ("dec3_w", (EP, EP)), ("xbar_w", (EP, NINP)),
        ("g1_w", (NINP, EP)), ("g2_w", (EP, EP)), ("g3_w", (EP, E3P)),
        ("g4_w", (E3P, NZP)), ("g6_w", (P, NINP)), ("g7_w", (NINP, EP)),
        ("g8_w", (EP, EP)), ("g9_w", (EP, NINP)),
        ("agnn_w", (CATP, NZP)), ("m1_w", (2 * EP, 2)), ("m2_w", (2 * EP, 2)),
        ("m3_w", (2 * E3P, 2)), ("ml_w", (CATP, 4)), ("wq", (P, NZP)),
    ]:
        wp[nm] = din(nm, sh)
    bp = {}
    for nm, sh in [
        ("enc1_b", (P, E_CH)), ("enc2_b", (P, E_CH)), ("enc3_b", (P, E3_CH)),
        ("zl_b", (NZP, 1)), ("dec1_b", (P, NIN_CH)), ("dec2_b", (P, E_CH)),
        ("dec3_b", (P, E_CH)), ("xbar_b", (P, NIN_CH)),
        ("m1_b", (2, 1)), ("m2_b", (2, 1)), ("m3_b", (2, 1)), ("ml_b", (4, 1)),
    ]:
        bp[nm] = din(nm, sh, F32)

    o_xbarT = dout("o_xbarT", [NINP, R])
    o_zhatT = dout("o_zhatT", [NINP, R])
    o_adjhat = dout("o_adjhat", [R, N])
    o_small = {nm: dout("o_" + nm, [R, NZP])
               for nm in ("q", "q1", "z", "r", "zl", "pred")}

    rg = [list(range(NCORES))]

    from contextlib import ExitStack
    with tile.TileContext(nc) as tc, ExitStack() as stack:
        pers = stack.enter_context(tc.tile_pool(name="pers", bufs=1))
        wst = stack.enter_context(tc.tile_pool(name="wst", bufs=3))
        tst = stack.enter_context(tc.tile_pool(name="tst", bufs=4))
        tmp = stack.enter_context(tc.tile_pool(name="tmp", bufs=2))
        sbst = stack.enter_context(tc.tile_pool(name="sbst", bufs=3))
        tlocp = stack.enter_context(tc.tile_pool(name="tlocp", bufs=1))
        ps = stack.enter_context(tc.tile_pool(name="ps", bufs=8, space="PSUM"))
        dram = stack.enter_context(tc.tile_pool(name="dram", bufs=1, space="DRAM"))

        _nmc = [0]

        def uname(pfx):
            _nmc[0] += 1
            return f"{pfx}{_nmc[0]}"

        # warmup: first collective pays ~50us of ncfw setup; hide it here
        wu_in = dram.tile([P, 16], BF, tag="wu_in", name="wu_in")
        wu_sb = pers.tile([P, 16], BF, tag="wu_sb")
        nc.gpsimd.memset(wu_sb[:], 0.0)
        nc.sync.dma_start(wu_in[:], wu_sb[:])
        wu_out = dram.tile([NCORES * P, 16], BF, tag="wu_out", name="wu_out",
                           addr_space="Shared")
        nc.gpsimd.collective_compute(
            "AllGather", ALU.bypass, replica_groups=rg,
            ins=[wu_in[:].opt()], outs=[wu_out[:].opt()])

        # ---------------- constants / params in SBUF ----------------
        ident = pers.tile([P, P], BF, tag="ident")
        make_identity(nc, ident[:])
        ones_f = pers.tile([P, 1], F32, tag="ones_f")
        nc.gpsimd.memset(ones_f[:], 1.0)
        ones_row = pers.tile([1, 512], BF, tag="ones_row")
        nc.gpsimd.memset(ones_row[:], 1.0)
        gt_e = pers.tile([P, R], F32, tag="gt_e")
        nc.gpsimd.memset(gt_e[:], 0.0)
        gt_s = pers.tile([P, R], F32, tag="gt_s")
        nc.gpsimd.memset(gt_s[:], 0.0)

        adjT = pers.tile([P, CC, R], BF, tag="adjT")
        nc.sync.dma_start(adjT[:], adjT_p.ap().rearrange("(po pi) f -> pi po f", pi=P))

        bias = {}
        for nm in bp:
            t = pers.tile(list(bp[nm].shape), F32, tag="b_" + nm, name="b_" + nm)
            nc.sync.dma_start(t[:], bp[nm].ap())
            bias[nm] = t

        def resident_w(nm, nch, nl):
            t = pers.tile([P, nch, nl], BF, tag="w_" + nm, name="w_" + nm)
            nc.sync.dma_start(t[:], wp[nm].ap().rearrange("(po pi) l -> pi po l", pi=P))
            return t

        m1w = resident_w("m1_w", 8, 2)
        m2w = resident_w("m2_w", 8, 2)
        m3w = resident_w("m3_w", 32, 2)
        mlw = resident_w("ml_w", CAT_CH, 4)
        agnnw = resident_w("agnn_w", CAT_CH, NZP)
        zlw = resident_w("zl_w", E3_CH, NZP)
        g4w = resident_w("g4_w", E3_CH, NZP)
        wqw = resident_w("wq", 1, NZP)
        dec1w = resident_w("dec1_w", 1, NINP)
        g6w = resident_w("g6_w", 1, NINP)

        # ---------------- helpers ----------------
        def wtile(param, k, f0, fsz):
            t = wst.tile([P, fsz], BF, tag="w", name=uname("wt"))
            nc.sync.dma_start(t[:], param.ap()[k * P:(k + 1) * P, f0:f0 + fsz])
            return t

        def psum(shape, dt=F32):
            return ps.tile(list(shape), dt, tag="ps", name=uname("ps"))

        def sb_chunk(src_dram, c):
            """stream one feature-major [128, R] bf16 chunk back from DRAM"""
            t = sbst.tile([P, R], BF, tag="sb", name=uname("sb"))
            nc.scalar.dma_start(t[:], src_dram[c * P:(c + 1) * P, :])
            return t[:]

        def fmajor_linear(dst, src_fn, wparam, nk, nfo_total, bias_t=None,
                          act=AF.Relu, dst_dram=None):
            """feature-major linear: out = act(W.T @ src + b).
            dst: [128, nfo, R] sbuf or None; dst_dram: [nfo*128, R] dram spill.
            src_fn(k) -> [128, R] AP."""
            nfo = nfo_total // P
            for g0 in range(0, nfo, 4):
                gn = min(4, nfo - g0)
                pss = [psum([P, R]) for _ in range(gn)]
                for k_ in range(nk):
                    wt = wtile(wparam, k_, g0 * P, gn * P)
                    src = src_fn(k_)
                    for j in range(gn):
                        nc.tensor.matmul(pss[j][:], wt[:, j * P:(j + 1) * P],
                                         src, start=(k_ == 0), stop=(k_ == nk - 1))
                for j in range(gn):
                    fo = g0 + j
                    b_ap = bias_t[:, fo:fo + 1] if bias_t is not None else 0.0
                    if dst is not None:
                        nc.scalar.activation(dst[:, fo, :], pss[j][:], act, bias=b_ap)
                    if dst_dram is not None:
                        ot = sbst.tile([P, R], BF, tag="sb", name=uname("fo"))
                        nc.scalar.activation(ot[:], pss[j][:], act, bias=b_ap)
                        nc.gpsimd.dma_start(dst_dram[fo * P:(fo + 1) * P, :], ot[:])

        def nmajor_linear(dst, src_fn, wparam, nk, fo_total, wtiles=None,
                          after_fog=None):
            """node-major inner product: dst [128, NB, fo_total] = src.T @ W.
            src_fn(k) -> feature-major [128, R] chunk. after_fog: {fog: cb}."""
            for fog in range((fo_total + 511) // 512):
                fsz = min(512, fo_total - fog * 512)
                pss = [psum([P, fsz]) for _ in range(NB)]
                for k_ in range(nk):
                    if wtiles is not None:
                        wt = wtiles[:, k_, fog * 512:fog * 512 + fsz]
                    else:
                        wt = wtile(wparam, k_, fog * 512, fsz)[:]
                    src = src_fn(k_)
                    for nb in range(NB):
                        nc.tensor.matmul(pss[nb][:], src[:, nb * P:(nb + 1) * P],
                                         wt, start=(k_ == 0), stop=(k_ == nk - 1))
                for nb in range(NB):
                    nc.vector.tensor_copy(dst[:, nb, fog * 512:fog * 512 + fsz],
                                          pss[nb][:])
                if after_fog and fog in after_fog:
                    after_fog[fog]()

        def allgather(loc, w, name, col0=0, wsub=None):
            """loc [128, NB, w] bf16 node-major -> gathered [N, wsub] dram,
            gathering only columns [col0, col0+wsub)."""
            wsub = wsub or w
            bounce = dram.tile([R, wsub], BF, tag="b_" + name, name="b_" + name)
            nc.sync.dma_start(bounce[:].rearrange("(po pi) f -> pi po f", pi=P),
                              loc[:, :, col0:col0 + wsub])
            full = dram.tile([N, wsub], BF, tag="g_" + name, name="g_" + name,
                             addr_space="Shared")
            nc.gpsimd.collective_compute(
                "AllGather", ALU.bypass, replica_groups=rg,
                ins=[bounce[:].opt()], outs=[full[:].opt()])
            return full

        def spmm_pass(pss, tfull, pw, f0=0):
            """one spmm accumulation pass over tfull cols [f0, f0+pw*128)."""
            for c2 in range(CC // 2):
                tt = tst.tile([P, 2, pw * P], BF, tag="t", name=uname("t"))
                nc.sync.dma_start(
                    tt[:], tfull[c2 * 2 * P:(c2 + 1) * 2 * P,
                                 f0:f0 + pw * P]
                    .rearrange("(po pi) f -> pi po f", pi=P))
                for i2 in range(2):
                    col = c2 * 2 + i2
                    for j in range(pw):
                        nc.tensor.matmul(pss[j][:], tt[:, i2, j * P:(j + 1) * P],
                                         adjT[:, col, :], start=(col == 0),
                                         stop=(col == CC - 1))

        def spmm_f(dstT, fulls, w_total, act, dst_dram=None, fp32_dram=None,
                   epilogue=None):
            """feature-major spmm over pass-aligned gathered tensors.
            fulls: list of (dram_tensor, pass_width_chunks)."""
            fo = 0
            for tfull, pw, f0 in fulls:
                pss = [psum([P, R]) for _ in range(pw)]
                spmm_pass(pss, tfull, pw, f0)
                for j in range(pw):
                    if fp32_dram is not None:
                        xo = tmp.tile([P, R], F32, tag="fo32", name=uname("fo32"),
                                      bufs=2)
                        nc.scalar.activation(xo[:], pss[j][:], act)
                        nc.sync.dma_start(fp32_dram[fo * P:(fo + 1) * P, :], xo[:])
                        if dstT is not None:
                            nc.vector.tensor_copy(dstT[:, fo, :], xo[:])
                    elif dstT is not None:
                        nc.scalar.activation(dstT[:, fo, :], pss[j][:], act)
                    elif dst_dram is not None:
                        ot = sbst.tile([P, R], BF, tag="sb", name=uname("so"))
                        nc.scalar.activation(ot[:], pss[j][:], act)
                        nc.gpsimd.dma_start(dst_dram[fo * P:(fo + 1) * P, :], ot[:])
                    fo += 1
                if epilogue is not None:
                    epilogue(fo)

        def spmm_narrow(tfull):
            """narrow spmm: load whole [N, NZP] in one DMA, one psum out."""
            tt = tst.tile([P, CC, NZP], BF, tag="tn", name=uname("tn"), bufs=2)
            nc.sync.dma_start(tt[:],
                              tfull[:].rearrange("(po pi) f -> pi po f", pi=P))
            pp = psum([NZP, R])
            for col in range(CC):
                nc.tensor.matmul(pp[:], tt[:, col, :], adjT[:, col, :],
                                 start=(col == 0), stop=(col == CC - 1))
            return pp

        def gate(src_list, wres, nktot, bias_t, nl):
            """normalize(softmax(lrelu(cat @ w + b))) row-L2; the softmax sum
            cancels under L2 normalization: P = e / ||e||_2.
            src_list: [(chunk_fn, nch)]; returns nl fp32 [128,R] bcast tiles."""
            pp = psum([nl, R])
            k_ = 0
            for fn, nch in src_list:
                for c in range(nch):
                    nc.tensor.matmul(pp[:], wres[:, k_, :nl], fn(c),
                                     start=(k_ == 0), stop=(k_ == nktot - 1))
                    k_ += 1
            assert k_ == nktot
            lin = tmp.tile([nl, R], F32, tag="gt_lin", name=uname("gt_lin"), bufs=1)
            nc.vector.tensor_scalar(lin[:], pp[:], bias_t[:, 0:1], None, ALU.add)
            lr = tmp.tile([nl, R], F32, tag="gt_lr", name=uname("gt_lr"), bufs=1)
            nc.vector.scalar_tensor_tensor(lr[:], lin[:], 0.01, lin[:],
                                           ALU.mult, ALU.max)
            nc.scalar.activation(gt_e[0:nl, :], lr[:], AF.Exp)
            sq = tmp.tile([nl, R], F32, tag="gt_sq", name=uname("gt_sq"), bufs=1)
            nc.vector.tensor_mul(sq[:], gt_e[0:nl, :], gt_e[0:nl, :])
            nc.vector.tensor_copy(gt_s[0:nl, :], sq[:])
            n2 = psum([1, R])
            nc.tensor.matmul(n2[:], ones_f[:, 0:1], gt_s[:], start=True, stop=True)
            sqr = tmp.tile([1, R], F32, tag="gt_sqr", name=uname("gt_sqr"), bufs=1)
            nc.scalar.activation(sqr[:], n2[:], AF.Sqrt)
            rn = tmp.tile([1, R], F32, tag="gt_rn", name=uname("gt_rn"), bufs=1)
            nc.vector.reciprocal(rn[:], sqr[:])
            rnb = tmp.tile([P, R], F32, tag="gt_rnb", name=uname("gt_rnb"), bufs=1)
            nc.gpsimd.partition_broadcast(rnb[:], rn[:])
            p_t = tmp.tile([nl, R], F32, tag="gt_p", name=uname("gt_p"), bufs=1)
            nc.vector.tensor_mul(p_t[:], gt_e[0:nl, :], rnb[0:nl, :])
            outs = []
            for i in range(nl):
                row = tmp.tile([1, R], F32, tag=f"gt_row{i}", name=uname("gt_row"),
                               bufs=1)
                if i == 0:
                    nc.vector.tensor_copy(row[:], p_t[0:1, :])
                else:
                    nc.sync.dma_start(row[:], p_t[i:i + 1, :])
                pb = tmp.tile([P, R], F32, tag=f"gt_pb{i}", name=uname("gt_pb"),
                              bufs=1)
                nc.gpsimd.partition_broadcast(pb[:], row[:])
                outs.append(pb)
            return outs

        def to_node_major(srcT, nch, dst):
            """srcT [128, nch, R] bf16 -> dst [128, NB, nch*128] node-major."""
            for fc in range(nch):
                for nb in range(NB):
                    ptr = psum([P, P], BF)
                    nc.tensor.transpose(ptr[:], srcT[:, fc, nb * P:(nb + 1) * P],
                                        ident[:])
                    nc.vector.tensor_copy(dst[:, nb, fc * P:(fc + 1) * P], ptr[:])

        def mix_chunk(a_ap, b_ap, pa, pb_, out_ap=None):
            """one [128,R] bf16 chunk of pa*a + pb*b"""
            t0 = tmp.tile([P, R], F32, tag="mx0", name=uname("mx0"))
            t1 = tmp.tile([P, R], F32, tag="mx1", name=uname("mx1"))
            if out_ap is None:
                out_ap = tmp.tile([P, R], BF, tag="mxo", name=uname("mxo"), bufs=1)[:]
            nc.vector.tensor_mul(t0[:], a_ap, pa[:])
            nc.vector.tensor_mul(t1[:], b_ap, pb_[:])
            nc.vector.tensor_add(out_ap, t0[:], t1[:])
            return out_ap

        def narrow_nm(srcT, bf_dst=None, f32_dram=None, want_f32=False):
            f32s = []
            for nb in range(NB):
                pt = psum([P, NZP], BF)
                nc.tensor.transpose(pt[:], srcT[0:NZP, nb * P:(nb + 1) * P],
                                    ident[0:NZP, 0:NZP])
                if bf_dst is not None:
                    nc.vector.tensor_copy(bf_dst[:, nb, :], pt[:])
                ft = None
                if want_f32 or f32_dram is not None:
                    ft = tmp.tile([P, NZP], F32, tag="nmf", name=uname("nmf"),
                                  bufs=NB + 1)
                    nc.vector.tensor_copy(ft[:], pt[:])
                    if f32_dram is not None:
                        nc.sync.dma_start(f32_dram[nb * P:(nb + 1) * P, :], ft[:])
                f32s.append(ft)
            return f32s

        def student_t(lhsT_aug, nm_f32s, out_dram):
            for nb in range(NB):
                pq = psum([P, NZP])
                nc.tensor.matmul(pq[:], lhsT_aug[:, nb * P:(nb + 1) * P],
                                 wqw[:, 0, :], start=True, stop=True)
                sqv = tmp.tile([P, NZ], F32, tag="q_sq", name=uname("q_sq"))
                nc.vector.tensor_mul(sqv[:], nm_f32s[nb][:, 0:NZ],
                                     nm_f32s[nb][:, 0:NZ])
                zn = tmp.tile([P, 1], F32, tag="q_zn", name=uname("q_zn"))
                nc.vector.tensor_reduce(zn[:], sqv[:], mybir.AxisListType.X, ALU.add)
                d2 = tmp.tile([P, NZ], F32, tag="q_d2", name=uname("q_d2"))
                nc.vector.tensor_scalar(d2[:], pq[:, 0:NZ], zn[:], None, ALU.add)
                qn = tmp.tile([P, NZ], F32, tag="q_qn", name=uname("q_qn"))
                nc.vector.reciprocal(qn[:], d2[:])
                sm = tmp.tile([P, 1], F32, tag="q_sm", name=uname("q_sm"))
                nc.vector.tensor_reduce(sm[:], qn[:], mybir.AxisListType.X, ALU.add)
                rs = tmp.tile([P, 1], F32, tag="q_rs", name=uname("q_rs"))
                nc.vector.reciprocal(rs[:], sm[:])
                ot = tmp.tile([P, NZP], F32, tag="q_ot", name=uname("q_ot"))
                nc.vector.memset(ot[:], 0.0)
                nc.vector.tensor_scalar(ot[:, 0:NZ], qn[:], rs[:], None, ALU.mult)
                nc.gpsimd.dma_start(out_dram[nb * P:(nb + 1) * P, :], ot[:])

        # ====================================================================
        # Schedule: the GNN chain is serial through 11 AllGathers; all
        # independent work (AE encoder/decoder, xbar, gates, student-t) is
        # placed to fill specific gather-latency windows.
        # ====================================================================

        # ---- phase A: t1 = x @ g1 first, gather ASAP ----
        t1_loc = tlocp.tile([P, NB, EP], BF, tag="tloc", name=uname("tloc"), bufs=2)
        ps_t1 = [psum([P, EP]) for _ in range(NB)]
        for k2 in range(NIN_CH // 2):
            xt2 = tst.tile([P, 2, R], BF, tag="t", name=uname("xt"))
            nc.sync.dma_start(xt2[:], xT_p.ap()[k2 * 2 * P:(k2 + 1) * 2 * P, :]
                              .rearrange("(po pi) f -> pi po f", pi=P))
            for i2 in range(2):
                k_ = k2 * 2 + i2
                g1t = wtile(wp["g1_w"], k_, 0, EP)
                for nb in range(NB):
                    nc.tensor.matmul(ps_t1[nb][:],
                                     xt2[:, i2, nb * P:(nb + 1) * P],
                                     g1t[:], start=(k_ == 0),
                                     stop=(k_ == NIN_CH - 1))
        for nb in range(NB):
            nc.vector.tensor_copy(t1_loc[:, nb, :], ps_t1[nb][:])
        t1_full = allgather(t1_loc, EP, "t1")

        # [AG-t1 window] enc1 + enc2
        r_e1T = pers.tile([P, E_CH, R], BF, tag="r_e1T")
        ps_e1 = [psum([P, R]) for _ in range(E_CH)]
        for k2 in range(NIN_CH // 2):
            xt2 = tst.tile([P, 2, R], BF, tag="t", name=uname("xt"))
            nc.sync.dma_start(xt2[:], xT_p.ap()[k2 * 2 * P:(k2 + 1) * 2 * P, :]
                              .rearrange("(po pi) f -> pi po f", pi=P))
            for i2 in range(2):
                k_ = k2 * 2 + i2
                e1t = wtile(wp["enc1_w"], k_, 0, EP)
                for fo in range(E_CH):
                    nc.tensor.matmul(ps_e1[fo][:], e1t[:, fo * P:(fo + 1) * P],
                                     xt2[:, i2, :], start=(k_ == 0),
                                     stop=(k_ == NIN_CH - 1))
        for fo in range(E_CH):
            nc.scalar.activation(r_e1T[:, fo, :], ps_e1[fo][:], AF.Relu,
                                 bias=bias["enc1_b"][:, fo:fo + 1])
        r_e2T = pers.tile([P, E_CH, R], BF, tag="r_e2T")
        fmajor_linear(r_e2T, lambda k: r_e1T[:, k, :], wp["enc2_w"], E_CH, EP,
                      bias["enc2_b"])

        # r_e3 = relu(enc3.T @ r_e2 + b) fused with r = r_e3 @ zl_w; split in
        # halves so the first two psum-groups fill the AG-t1 window.
        _pr_box = []

        def re3_groups(g0s):
            if not _pr_box:
                _pr_box.append(psum([NZP, R]))
            pr = _pr_box[0]
            for g0 in g0s:
                pss = [psum([P, R]) for _ in range(4)]
                for k_ in range(E_CH):
                    wt = wtile(wp["enc3_w"], k_, g0 * P, 4 * P)
                    for j in range(4):
                        nc.tensor.matmul(pss[j][:], wt[:, j * P:(j + 1) * P],
                                         r_e2T[:, k_, :], start=(k_ == 0),
                                         stop=(k_ == E_CH - 1))
                for j in range(4):
                    fo = g0 + j
                    d3t = sbst.tile([P, R], BF, tag="sb", name=uname("re3"))
                    nc.scalar.activation(d3t[:], pss[j][:], AF.Relu,
                                         bias=bias["enc3_b"][:, fo:fo + 1])
                    nc.tensor.matmul(pr[:], zlw[:, fo, :], d3t[:],
                                     start=(fo == 0), stop=(fo == E3_CH - 1))

        re3_groups([0, 4])

        # ---- z1 ----
        z1T = pers.tile([P, E_CH, R], BF, tag="z1T")
        spmm_f(z1T, [(t1_full, 4, 0)], EP, AF.Relu)
        p1 = gate([(lambda c: r_e1T[:, c, :], E_CH), (lambda c: z1T[:, c, :], E_CH)],
                  m1w, 8, bias["m1_b"], 2)
        t2_loc = tlocp.tile([P, NB, EP], BF, tag="tloc", name=uname("tloc"), bufs=2)
        nmajor_linear(t2_loc, lambda k: mix_chunk(z1T[:, k, :], r_e1T[:, k, :],
                                                  p1[0], p1[1]),
                      wp["g2_w"], E_CH, EP)
        t2_full = allgather(t2_loc, EP, "t2")

        # r_e3 (fused with r = r_e3 @ zl_w + zl_b): second half
        re3_groups([8, 12])
        rT = pers.tile([P, R], BF, tag="rT")
        nc.gpsimd.memset(rT[:], 0.0)
        nc.scalar.activation(rT[0:NZP, :], _pr_box[0][:], AF.Identity,
                             bias=bias["zl_b"][:, 0:1])
        nc.sync.dma_start(rT[NZ:NZ + 1, :], ones_row[0:1, 0:R])  # student-t aug
        r_nm_bf = pers.tile([P, NB, NZP], BF, tag="r_nm")
        r_f32s = narrow_nm(rT, bf_dst=r_nm_bf, f32_dram=o_small["r"].ap(),
                           want_f32=True)
        r_full = allgather(r_nm_bf, NZP, "r")
        student_t(rT, r_f32s, o_small["q1"].ap())

        # ---- z2 ----
        z2T = pers.tile([P, E_CH, R], BF, tag="z2T")
        spmm_f(z2T, [(t2_full, 4, 0)], EP, AF.Relu)
        # associativity: z3 = relu((adj @ z2) @ g3) -- gather narrow z2 (4MB)
        # instead of the 2000-wide z2@g3 (16MB), and spmm over 512 not 2048.
        z2nm = tlocp.tile([P, NB, EP], BF, tag="tloc", name=uname("z2nm"), bufs=2)
        to_node_major(z2T, E_CH, z2nm)
        z2_full = allgather(z2nm, EP, "z2")

        # [AG-z2 window] m2 gate, mix2, h3 -> DRAM
        p2 = gate([(lambda c: r_e2T[:, c, :], E_CH), (lambda c: z2T[:, c, :], E_CH)],
                  m2w, 8, bias["m2_b"], 2)
        h3_d = dram.tile([E3P, R], BF, tag="h3_d", name="h3_d")
        mix2T = pers.tile([P, E_CH, R], BF, tag="mix2T")
        for c in range(E_CH):
            mix_chunk(z2T[:, c, :], r_e2T[:, c, :], p2[0], p2[1],
                      out_ap=mix2T[:, c, :])
        fmajor_linear(None, lambda k: mix2T[:, k, :], wp["enc3_w"],
                      E_CH, E3P, bias["enc3_b"], dst_dram=h3_d[:])

        # u2 = adj @ z2 (raw), then z3 = relu(u2 @ g3) -> DRAM
        u2T = pers.tile([P, E_CH, R], BF, tag="uT", name="u2T")
        spmm_f(u2T, [(z2_full, 4, 0)], EP, AF.Copy)
        z3_d = dram.tile([E3P, R], BF, tag="z3_d", name="z3_d")
        fmajor_linear(None, lambda k: u2T[:, k, :], wp["g3_w"], E_CH, E3P,
                      None, AF.Relu, dst_dram=z3_d[:])


        # ---- m3 gate, t4 = mix3 @ g4 ----
        p3 = gate([(lambda c: sb_chunk(h3_d[:], c), E3_CH),
                   (lambda c: sb_chunk(z3_d[:], c), E3_CH)],
                  m3w, 32, bias["m3_b"], 2)
        t4_loc = pers.tile([P, NB, NZP], BF, tag="t4_loc")
        nmajor_linear(t4_loc, lambda k: mix_chunk(sb_chunk(z3_d[:], k),
                                                  sb_chunk(h3_d[:], k),
                                                  p3[0], p3[1]),
                      wp["g4_w"], E3_CH, NZP, wtiles=g4w)
        t4_full = allgather(t4_loc, NZP, "t4")
        xbar_group(0)

        # [AG-t4 window] dec1+dec2 fused
        r_d2_d = dram.tile([EP, R], BF, tag="r_d2_d", name="r_d2_d")
        ps_d2 = [psum([P, R]) for _ in range(E_CH)]
        for k_ in range(NIN_CH):
            pd1 = psum([P, R])
            nc.tensor.matmul(pd1[:], dec1w[:, 0, k_ * P:(k_ + 1) * P], rT[:],
                             start=True, stop=True)
            d1t = tmp.tile([P, R], BF, tag="d1t", name=uname("d1t"))
            nc.scalar.activation(d1t[:], pd1[:], AF.Relu,
                                 bias=bias["dec1_b"][:, k_:k_ + 1])
            w2t = wtile(wp["dec2_w"], k_, 0, EP)
            for fo in range(E_CH):
                nc.tensor.matmul(ps_d2[fo][:], w2t[:, fo * P:(fo + 1) * P], d1t[:],
                                 start=(k_ == 0), stop=(k_ == NIN_CH - 1))
        for fo in range(E_CH):
            d2o = sbst.tile([P, R], BF, tag="sb", name=uname("d2o"))
            nc.scalar.activation(d2o[:], ps_d2[fo][:], AF.Relu,
                                 bias=bias["dec2_b"][:, fo:fo + 1])
            nc.gpsimd.dma_start(r_d2_d[:][fo * P:(fo + 1) * P, :], d2o[:])

        # ---- z = relu(adj @ t4), node-major z; gather z (tiny) ----
        pz = spmm_narrow(t4_full)
        zT = pers.tile([P, R], BF, tag="zT")
        nc.gpsimd.memset(zT[:], 0.0)
        nc.scalar.activation(zT[0:NZP, :], pz[:], AF.Relu)
        z_nm_bf = pers.tile([P, NB, NZP], BF, tag="z_nm")
        for nb in range(NB):
            pt = psum([P, NZP], BF)
            nc.tensor.transpose(pt[:], zT[0:NZP, nb * P:(nb + 1) * P],
                                ident[0:NZP, 0:NZP])
            nc.vector.tensor_copy(z_nm_bf[:, nb, :], pt[:])
            zf = tmp.tile([P, NZP], F32, tag="nmf", name=uname("zf"), bufs=NB + 1)
            nc.vector.tensor_copy(zf[:], pt[:])
            nc.gpsimd.dma_start(o_small["z"].ap()[nb * P:(nb + 1) * P, :], zf[:])
        z_full = allgather(z_nm_bf, NZP, "z")

        # [AG-z window] ml gate + t_a + its gather
        zT3 = zT.rearrange("p (c f) -> p c f", c=1)
        pml = gate([(lambda c: z1T[:, c, :], E_CH), (lambda c: z2T[:, c, :], E_CH),
                    (lambda c: sb_chunk(z3_d[:], c), E3_CH),
                    (lambda c: zT3[:, 0, :], 1)],
                   mlw, CAT_CH, bias["ml_b"], 4)
        ta_loc = pers.tile([P, NB, NZP], BF, tag="ta_loc")
        ps_ta = [psum([P, NZP]) for _ in range(NB)]
        blocks = [(lambda c: z1T[:, c, :], E_CH, pml[0]),
                  (lambda c: z2T[:, c, :], E_CH, pml[1]),
                  (lambda c: sb_chunk(z3_d[:], c), E3_CH, pml[2]),
                  (lambda c: zT3[:, 0, :], 1, pml[3])]
        k_ = 0
        for fn, nch, pb_ in blocks:
            for c in range(nch):
                mz = tmp.tile([P, R], BF, tag="mz", name=uname("mz"), bufs=1)
                nc.vector.tensor_mul(mz[:], fn(c), pb_[:])
                for nb in range(NB):
                    nc.tensor.matmul(ps_ta[nb][:], mz[:, nb * P:(nb + 1) * P],
                                     agnnw[:, k_, :], start=(k_ == 0),
                                     stop=(k_ == CAT_CH - 1))
                k_ += 1
        for nb in range(NB):
            nc.vector.tensor_copy(ta_loc[:, nb, :], ps_ta[nb][:])
        ta_full = allgather(ta_loc, NZP, "ta")

        # az = adj@z, ar = adj@r (raw); z_l = az + ar; dz1 = relu(az @ g6)
        paz = spmm_narrow(z_full)
        azT = pers.tile([P, R], BF, tag="azT")
        nc.gpsimd.memset(azT[:], 0.0)
        nc.vector.tensor_copy(azT[0:NZP, :], paz[:])
        par = spmm_narrow(r_full)
        z_lT = pers.tile([P, R], BF, tag="z_lT")
        nc.gpsimd.memset(z_lT[:], 0.0)
        nc.vector.tensor_add(z_lT[0:NZP, :], azT[0:NZP, :], par[:])
        nc.sync.dma_start(z_lT[NZ:NZ + 1, :], ones_row[0:1, 0:R])
        # dz1 = relu((adj @ z) @ g6), feature-major, kept in SBUF
        dz1T = pers.tile([P, NIN_CH, R], BF, tag="dz1T")
        for fo in range(NIN_CH):
            pd = psum([P, R])
            nc.tensor.matmul(pd[:], g6w[:, 0, fo * P:(fo + 1) * P], azT[:],
                             start=True, stop=True)
            nc.scalar.activation(dz1T[:, fo, :], pd[:], AF.Relu)
        t7_loc = tlocp.tile([P, NB, EP], BF, tag="tloc", name=uname("tloc"), bufs=2)
        nmajor_linear(t7_loc, lambda k: dz1T[:, k, :], wp["g7_w"],
                      NIN_CH, EP)
        t7_full = allgather(t7_loc, EP, "t7")
        dec12_half(0)

        # [AG-t7 window] pred = softmax(adj @ t_a)
        tta = tst.tile([P, CC, NZP], BF, tag="tn", name=uname("tta"), bufs=2)
        nc.sync.dma_start(tta[:],
                          ta_full[:].rearrange("(po pi) f -> pi po f", pi=P))
        ps_pred = [psum([P, NZP]) for _ in range(NB)]
        for col in range(CC):
            for nb in range(NB):
                nc.tensor.matmul(ps_pred[nb][:], adjT[:, col, nb * P:(nb + 1) * P],
                                 tta[:, col, :], start=(col == 0),
                                 stop=(col == CC - 1))
        for nb in range(NB):
            ex = tmp.tile([P, NZ], F32, tag="pr_e", name=uname("pr_e"))
            s = tmp.tile([P, 1], F32, tag="pr_s", name=uname("pr_s"))
            nc.scalar.activation(ex[:], ps_pred[nb][:, 0:NZ], AF.Exp, accum_out=s[:])
            ri = tmp.tile([P, 1], F32, tag="pr_ri", name=uname("pr_ri"))
            nc.vector.reciprocal(ri[:], s[:])
            po = tmp.tile([P, NZP], F32, tag="pr_o", name=uname("pr_o"))
            nc.vector.memset(po[:], 0.0)
            nc.vector.tensor_scalar(po[:, 0:NZ], ex[:], ri[:], None, ALU.mult)
            nc.gpsimd.dma_start(o_small["pred"].ap()[nb * P:(nb + 1) * P, :], po[:])


        xbar_group(8)

        # ---- dz2 ----
        dz2T = pers.tile([P, E_CH, R], BF, tag="dz2T")
        spmm_f(dz2T, [(t7_full, 4, 0)], EP, AF.Relu)
        t8_loc = tlocp.tile([P, NB, EP], BF, tag="tloc", name=uname("tloc"), bufs=2)
        nmajor_linear(t8_loc, lambda k: dz2T[:, k, :], wp["g8_w"], E_CH, EP)
        t8_full = allgather(t8_loc, EP, "t8")

        # [AG-t8 window] student-t q on z_l
        zl_f32s = narrow_nm(z_lT, f32_dram=o_small["zl"].ap(), want_f32=True)
        student_t(z_lT, zl_f32s, o_small["q"].ap())


        # ---- dz3 ----
        dz3T = pers.tile([P, E_CH, R], BF, tag="dz3T")
        spmm_f(dz3T, [(t8_full, 4, 0)], EP, AF.Relu)
        # associativity: z_hat = relu((adj @ dz3) @ g9)
        dz3nm = tlocp.tile([P, NB, EP], BF, tag="tloc", name=uname("dz3nm"), bufs=2)
        to_node_major(dz3T, E_CH, dz3nm)
        dz3_full = allgather(dz3nm, EP, "dz3")

        u9T = pers.tile([P, E_CH, R], BF, tag="uT", name="u9T")
        spmm_f(u9T, [(dz3_full, 4, 0)], EP, AF.Copy)

        # z_hat = relu(u9 @ g9): feature-major groups, zh gather per quarter
        z_hatT = pers.tile([P, NIN_CH, R], BF, tag="z_hatT")
        zh_bounce = dram.tile([NINP, R], BF, tag="b_zh", name="b_zh")
        zh_h = []
        for g0 in range(0, NIN_CH, 4):
            pss = [psum([P, R]) for _ in range(4)]
            for k_ in range(E_CH):
                wt = wtile(wp["g9_w"], k_, g0 * P, 4 * P)
                for j in range(4):
                    nc.tensor.matmul(pss[j][:], wt[:, j * P:(j + 1) * P],
                                     u9T[:, k_, :], start=(k_ == 0),
                                     stop=(k_ == E_CH - 1))
            for j in range(4):
                fo = g0 + j
                xo = tmp.tile([P, R], F32, tag="fo32", name=uname("fo32"), bufs=2)
                nc.scalar.activation(xo[:], pss[j][:], AF.Relu)
                nc.gpsimd.dma_start(o_zhatT.ap()[fo * P:(fo + 1) * P, :], xo[:])
                nc.vector.tensor_copy(z_hatT[:, fo, :], xo[:])
                nc.sync.dma_start(zh_bounce[fo * P:(fo + 1) * P, :],
                                   z_hatT[:, fo, :])
            p_i = g0 // 4
            full = dram.tile([NCORES * 512, R], BF, tag=f"g_zh{p_i}",
                             name=f"g_zh{p_i}", addr_space="Shared")
            nc.gpsimd.collective_compute(
                "AllGather", ALU.bypass, replica_groups=rg,
                ins=[zh_bounce[p_i * 512:(p_i + 1) * 512, :].opt()],
                outs=[full[:].opt()])
            zh_h.append(full)

        xbar_group(0)
        xbar_group(4)
        xbar_group(8)
        xbar_group(12)

        # ---- adj_hat = sigmoid(z_hat @ z_hat^T) ----
        for cb in range(NCORES):
            pss = [psum([P, R]) for _ in range(NB)]
            for f2 in range(NIN_CH // 2):
                h = f2 // 2
                base = cb * 512 + (f2 % 2) * 256
                rt2 = tst.tile([P, 2, R], BF, tag="t", name=uname("rt"))
                nc.sync.dma_start(rt2[:], zh_h[h][base:base + 256, :]
                                  .rearrange("(po pi) f -> pi po f", pi=P))
                for i2 in range(2):
                    f = f2 * 2 + i2
                    for nb in range(NB):
                        nc.tensor.matmul(pss[nb][:],
                                         z_hatT[:, f, nb * P:(nb + 1) * P],
                                         rt2[:, i2, :], start=(f == 0),
                                         stop=(f == NIN_CH - 1))
            for nb in range(NB):
                so = tmp.tile([P, R], F32, tag="fo32", name=uname("ah"), bufs=2)
                nc.scalar.activation(so[:], pss[nb][:], AF.Sigmoid)
                nc.gpsimd.dma_start(o_adjhat.ap()[nb * P:(nb + 1) * P,
                                                cb * R:(cb + 1) * R], so[:])

    nc.compile()
    return nc


# ----------------------------------------------------------------------------
# host-side input prep
# ----------------------------------------------------------------------------

def _pad2(a, s0, s1):
    z = np.zeros((s0, s1), np.float32)
    z[:a.shape[0], :a.shape[1]] = a
    return z


def prep_in_maps(inputs, N=N_FULL):
    R = N // NCORES
    f32 = {k: np.asarray(v, np.float32) for k, v in inputs.items()}

    shared = {}
    for nm, s0, s1 in [
        ("enc1_w", NINP, EP), ("enc2_w", EP, EP), ("enc3_w", EP, E3P),
        ("zl_w", E3P, NZP), ("dec1_w", P, NINP), ("dec2_w", NINP, EP),
        ("dec3_w", EP, EP), ("xbar_w", EP, NINP),
        ("g1_w", NINP, EP), ("g2_w", EP, EP), ("g3_w", EP, E3P),
        ("g4_w", E3P, NZP), ("g6_w", P, NINP), ("g7_w", NINP, EP),
        ("g8_w", EP, EP), ("g9_w", EP, NINP),
    ]:
        shared[nm] = _pad2(f32[nm], s0, s1).astype(bf16)

    def blocks2(w, bsz, bpad):
        nb_ = w.shape[0] // bsz
        out = np.zeros((nb_ * bpad, w.shape[1]), np.float32)
        for i in range(nb_):
            out[i * bpad:i * bpad + bsz] = w[i * bsz:(i + 1) * bsz]
        return out

    shared["m1_w"] = blocks2(f32["m1_w"], E1, EP).astype(bf16)
    shared["m2_w"] = blocks2(f32["m2_w"], E1, EP).astype(bf16)
    shared["m3_w"] = blocks2(f32["m3_w"], E3, E3P).astype(bf16)

    def blocks_cat(w, ncols=None):
        out = np.zeros((CATP, ncols or w.shape[1]), np.float32)
        out[0:E1, :w.shape[1]] = w[0:E1]
        out[EP:EP + E1, :w.shape[1]] = w[E1:2 * E1]
        out[2 * EP:2 * EP + E3, :w.shape[1]] = w[2 * E1:2 * E1 + E3]
        out[2 * EP + E3P:2 * EP + E3P + NZ, :w.shape[1]] = w[2 * E1 + E3:2 * E1 + E3 + NZ]
        return out

    shared["ml_w"] = blocks_cat(f32["ml_w"]).astype(bf16)
    shared["agnn_w"] = blocks_cat(f32["agnn_w"], NZP).astype(bf16)

    cl = f32["cluster"]                      # [K, NZ]
    wq = np.zeros((P, NZP), np.float32)
    wq[0:NZ, 0:K] = -2.0 * cl.T
    wq[NZ, 0:K] = (cl * cl).sum(axis=1) + 1.0
    shared["wq"] = wq.astype(bf16)

    for nm, total, nch in [
        ("enc1_b", EP, E_CH), ("enc2_b", EP, E_CH), ("enc3_b", E3P, E3_CH),
        ("dec1_b", NINP, NIN_CH), ("dec2_b", EP, E_CH), ("dec3_b", EP, E_CH),
        ("xbar_b", NINP, NIN_CH),
    ]:
        b = np.zeros(total, np.float32)
        b[:f32[nm].shape[0]] = f32[nm]
        shared[nm] = np.ascontiguousarray(b.reshape(nch, P).T)
    shared["zl_b"] = _pad2(f32["zl_b"][:, None], NZP, 1)
    for nm, nl in [("m1_b", 2), ("m2_b", 2), ("m3_b", 2), ("ml_b", 4)]:
        shared[nm] = np.ascontiguousarray(f32[nm].reshape(nl, 1))

    adjT = np.ascontiguousarray(f32["adj"].T).astype(bf16)      # [N, N]
    xT = np.zeros((NINP, N), np.float32)
    xT[0:NIN] = f32["x"].T
    xT = xT.astype(bf16)

    in_maps = []
    for c in range(NCORES):
        m = dict(shared)
        m["adjT"] = np.ascontiguousarray(adjT[:, c * R:(c + 1) * R])
        m["xT"] = np.ascontiguousarray(xT[:, c * R:(c + 1) * R])
        in_maps.append(m)
    return in_maps


def assemble_outputs(results, N=N_FULL):
    def cat_rows(key):
        return np.concatenate([r[key] for r in results], axis=0)

    x_bar = np.concatenate([r["o_xbarT"][0:NIN, :].T for r in results], axis=0)
    z_hat = np.concatenate([r["o_zhatT"][0:NIN, :].T for r in results], axis=0)
    adj_hat = cat_rows("o_adjhat")
    q = cat_rows("o_q")[:, 0:NZ]
    q1 = cat_rows("o_q1")[:, 0:NZ]
    z = cat_rows("o_z")[:, 0:NZ]
    r_ = cat_rows("o_r")[:, 0:NZ]
    z_l = cat_rows("o_zl")[:, 0:NZ]
    pred = cat_rows("o_pred")[:, 0:NZ]
    return (x_bar, z_hat, adj_hat, q, q1, z, r_, z_l, pred)


def _run(inputs, trace=False):
    if N_FULL not in _BUILD_CACHE:
        _BUILD_CACHE[N_FULL] = build_graph(N_FULL)
    nc = _BUILD_CACHE[N_FULL]
    in_maps = prep_in_maps(inputs, N_FULL)
    res = run_bass_kernel_spmd(nc, in_maps, list(range(NCORES)), trace=trace)
    outs = assemble_outputs(res.results, N_FULL)
    return outs, res


def kernel(**inputs):
    outs, _ = _run(inputs, trace=False)
    return outs
